# revision 26
# baseline (speedup 1.0000x reference)
"""Trainium2 Bass kernel for gated multi-head attention (nn_MHAtt_41274635714591).

Strategy: data-parallel over batch — 8 batches onto 8 NeuronCores, one batch per
core, no collectives. Per core (S=1024, D=1024, H=8, DB=128):

  1. Inputs f32->bf16 cast on ACT — NOT gpsimd (3.2x slower; was the top
     bottleneck: PE idled ~25us at each input phase start waiting on casts).
     128x128 transposes on PE -> xT [d, s].
  2. Projections (bf16 matmuls, fp32 PSUM): qhT/khT = (x @ W + b)^T via
     lhsT=W-colblock, rhs=xT; vh in natural [s, d] layout straight into
     vh_aug whose extra all-ones column yields the softmax denominator
     for free from the PV matmul. Weights stream as 2MB column-halves on
     the gpsimd DMA queue (so they never head-of-line-block the x rows on
     the sync queue); casts: Wq on DVE (startup), Wk/Wv on ACT (slack in
     the q/k phases), Wm on gpsimd (during the ACT-bound attention phase).
  3. Gate MLP per head: gx = psx + bgX on DVE; tt = (psy + bgY) * gx in one
     DVE scalar_tensor_tensor; sigmoid on ACT. Gate rows are produced already
     broadcast across partitions by replicating the Wg2 column across the
     matmul's stationary dim; gates multiply khT/qhT in place.
  4. Scores computed TRANSPOSED: S^T[k,q] = lhsT=khT-chunk, rhs=qhT.
     exp(scale*x + maskbias_k) on ACT writes P^T directly — no P transposes.
     The mask folds in as a per-partition additive -1e9 bias. Heads 0/1
     pulled into the v phase (m=1,3) and head 2 right after, so ACT's exp
     stream (the attention-phase bottleneck at ~8.9us/head) starts ~3 heads
     early.
  5. PV: out[q, 0:129] = sum_k P^T-chunk^T @ vh_aug; column 128 is the
     denominator; normalize with DVE reciprocal + tensor_scalar.
  6. att tiles transposed on PE into A_T [d, s]; merge matmul with Wm
     col-halves streamed during the attention loop; + bm; DMA out.

The harness calls kernel(**full_inputs); we shard batch across cores with
run_bass_kernel_spmd and stack the per-core outputs.
"""

import math
import os
import sys

for _p in ("/opt/trn_rl_repo", "/root/.axon_site/_ro/trn_rl_repo"):
    if os.path.isdir(_p) and _p not in sys.path:
        sys.path.insert(0, _p)

import numpy as np

import concourse.bass as bass
import concourse.mybir as mybir
import concourse.tile as tile
from concourse import bacc
from concourse.masks import make_identity

F32 = mybir.dt.float32
BF16 = mybir.dt.bfloat16
U8 = mybir.dt.uint8
AF = mybir.ActivationFunctionType
OP = mybir.AluOpType

B, S, D, H = 8, 1024, 1024, 8
DB = D // H          # 128 per-head dim
P = 128              # partitions
KJ = S // P          # 8 tiles of 128 along s
NDT = D // P         # 8 tiles of 128 along d
SCALE = 1.0 / math.sqrt(DB)
NEG = -1e9


def build_nc(proj_bf16=True, attn_bf16=True, repeat=1):
    """Emit the per-core program. repeat>1 wraps the whole body in a
    device-side loop (for timing)."""
    assert proj_bf16 and attn_bf16
    pdt = BF16
    adt = BF16
    # Bacc (not plain Bass): its compile pipeline fuses multi-sem waits into
    # event semaphores — this container's walrus rejects instructions carrying
    # more than one sync wait — and inserts GPSIMD library / ACT table loads.
    nc = bacc.Bacc()

    q = nc.dram_tensor("q", [S, D], F32, kind="ExternalInput")
    k = nc.dram_tensor("k", [S, D], F32, kind="ExternalInput")
    v = nc.dram_tensor("v", [S, D], F32, kind="ExternalInput")
    mask = nc.dram_tensor("mask", [S], U8, kind="ExternalInput")
    Wq = nc.dram_tensor("Wq", [D, D], F32, kind="ExternalInput")
    Wk = nc.dram_tensor("Wk", [D, D], F32, kind="ExternalInput")
    Wv = nc.dram_tensor("Wv", [D, D], F32, kind="ExternalInput")
    Wm = nc.dram_tensor("Wm", [D, D], F32, kind="ExternalInput")
    bq = nc.dram_tensor("bq", [D], F32, kind="ExternalInput")
    bk = nc.dram_tensor("bk", [D], F32, kind="ExternalInput")
    bv = nc.dram_tensor("bv", [D], F32, kind="ExternalInput")
    bm = nc.dram_tensor("bm", [D], F32, kind="ExternalInput")
    WgX = nc.dram_tensor("WgX", [DB, DB], F32, kind="ExternalInput")
    WgY = nc.dram_tensor("WgY", [DB, DB], F32, kind="ExternalInput")
    Wg2 = nc.dram_tensor("Wg2", [DB, 2], F32, kind="ExternalInput")
    bgX = nc.dram_tensor("bgX", [DB], F32, kind="ExternalInput")
    bgY = nc.dram_tensor("bgY", [DB], F32, kind="ExternalInput")
    bg2 = nc.dram_tensor("bg2", [2], F32, kind="ExternalInput")
    out = nc.dram_tensor("out", [S, D], F32, kind="ExternalOutput")

    from contextlib import ExitStack

    with tile.TileContext(nc) as tc, ExitStack() as ctx:
        consts = ctx.enter_context(tc.tile_pool(name="consts", bufs=1))
        persist = ctx.enter_context(tc.tile_pool(name="persist", bufs=1))
        big = ctx.enter_context(tc.tile_pool(name="big", bufs=4))
        xrow = ctx.enter_context(tc.tile_pool(name="xrow", bufs=5))
        xbrow = ctx.enter_context(tc.tile_pool(name="xbrow", bufs=2))
        wstream = ctx.enter_context(tc.tile_pool(name="wstream", bufs=3))
        wconv = ctx.enter_context(tc.tile_pool(name="wconv", bufs=2))
        gpool = ctx.enter_context(tc.tile_pool(name="gpool", bufs=2))
        attp = ctx.enter_context(tc.tile_pool(name="attp", bufs=2))
        smalls = ctx.enter_context(tc.tile_pool(name="smalls", bufs=2))
        outp = ctx.enter_context(tc.tile_pool(name="outp", bufs=2))
        brep = ctx.enter_context(tc.tile_pool(name="brep", bufs=1))
        # PSUM: psc 2x[128,1024]f32 (4 banks) + ppv 2x[128,129]f32 (2 banks)
        # + ptr 2x[128,1024]bf16 (2 banks) = 8 banks
        psc = ctx.enter_context(tc.tile_pool(name="psc", bufs=2, space="PSUM"))
        ppv = ctx.enter_context(tc.tile_pool(name="ppv", bufs=2, space="PSUM"))
        ptr = ctx.enter_context(tc.tile_pool(name="ptr", bufs=2, space="PSUM"))

        # ---- identity + persistent activations (identity + the vh_aug ones
        # column are input-independent — emitted before the timing loop) ----
        identp = consts.tile([P, P], pdt, tag="identp")
        make_identity(nc, identp)

        qhT = persist.tile([P, H, S], adt, tag="qhT")   # [db, h, s] = (q@Wq+b)^T
        khT = persist.tile([P, H, S], adt, tag="khT")
        vh_aug = persist.tile([P, H, KJ, DB + 1], adt, tag="vh_aug")
        nc.vector.memset(vh_aug[:, :, :, DB : DB + 1], 1.0)
        A_T = persist.tile([P, H, S], pdt, tag="A_T")   # attention out, transposed

        if repeat > 1:
            ctx.enter_context(tc.For_i(0, repeat, 1))

        def cast(eng, dst, src):
            if eng is nc.scalar:
                nc.scalar.copy(dst, src)
            else:
                eng.tensor_copy(dst, src)

        # ---- input transpose: x [s, d] -> xT [d-in-tile, i, s] (bf16) ----
        # Each 128-row block loads as TWO half-row DMAs fired simultaneously
        # on the sync and scalar HWDGE queues (per-queue descriptor
        # generation is the DMA latency driver, so splitting halves the
        # arrival time), one block ahead of the cast/transpose consumption.
        def load_xT(xdram, ceng):
            xT = big.tile([P, NDT, S], pdt, tag="bigslab")
            xfs = {}

            def issue(m):
                if m >= KJ:
                    return
                hs = []
                for half, deng in ((0, nc.sync), (1, nc.scalar)):
                    xf = xrow.tile([P, 512], F32, tag="xrow")
                    deng.dma_start(
                        out=xf,
                        in_=xdram[m * P : (m + 1) * P,
                                  half * 512 : (half + 1) * 512],
                    )
                    hs.append(xf)
                xfs[m] = hs

            issue(0)
            issue(1)
            for m in range(KJ):
                xb = xbrow.tile([P, D], pdt, tag="xbrow")
                halves = xfs.pop(m)
                pt = ptr.tile([P, NDT * P], pdt, tag="trps")
                for half in range(2):
                    sl = slice(half * 512, (half + 1) * 512)
                    cast(ceng, xb[:, sl], halves[half])
                    for dj in range(half * 4, half * 4 + 4):
                        nc.tensor.transpose(
                            pt[:, dj * P : (dj + 1) * P],
                            xb[:, dj * P : (dj + 1) * P],
                            identp,
                        )
                issue(m + 2)
                # copy out in halves: with subtile deps, the next block's
                # transposes can reuse the first half of the pt slot sooner
                ptv = pt.rearrange("p (a b) -> p a b", b=P)
                nc.vector.tensor_copy(
                    xT[:, 0:4, m * P : (m + 1) * P], ptv[:, 0:4, :]
                )
                nc.vector.tensor_copy(
                    xT[:, 4:8, m * P : (m + 1) * P], ptv[:, 4:8, :]
                )
            return xT

        def load_w(Wdram, ceng):
            """Stream W in column-half order ([D, 512] then [D, 512]) as 2KB
            descriptor chunks on the gpsimd SWDGE queue: the projections'
            first accumulation sweep only needs half-0 chunk i=0, and half-1
            streams while half-0 is being consumed."""
            wbs = []
            for half in range(2):
                wb = wconv.tile(
                    [P, NDT, 512], pdt, tag=f"wchb{half}", name="wb"
                )
                wsrc = Wdram[:, half * 512 : (half + 1) * 512].rearrange(
                    "(i p) n -> p i n", p=P
                )
                for c in range(0, NDT, 2):
                    wf = wstream.tile([P, 2, 512], F32, tag="wch")
                    nc.gpsimd.dma_start(out=wf, in_=wsrc[:, c : c + 2, :])
                    cast(ceng, wb[:, c : c + 2, :], wf)
                wbs.append(wb)
            return wbs[0], wbs[1]

        # ---- startup: critical DMAs first (Wq chunks on the gpsimd queue,
        # q half-rows on the sync+scalar queues — all stream concurrently) ----
        wq0, wq1 = load_w(Wq, nc.vector)
        xTq = load_xT(q, nc.scalar)

        # ---- constants / small prep (nothing here is needed before ~15us;
        # emitted after the startup DMAs so the tiny SWDGE loads don't delay
        # them in the queues) ----
        with nc.allow_non_contiguous_dma(reason="tiny partition-major loads"):
            mask_u8 = consts.tile([P, KJ], U8, tag="mask_u8")
            nc.gpsimd.dma_start(
                out=mask_u8, in_=mask.rearrange("(o p) -> p o", p=P)
            )
            bq_sb = consts.tile([P, NDT], F32, tag="bq_sb")
            nc.gpsimd.dma_start(out=bq_sb, in_=bq.rearrange("(o p) -> p o", p=P))
            bk_sb = consts.tile([P, NDT], F32, tag="bk_sb")
            nc.gpsimd.dma_start(out=bk_sb, in_=bk.rearrange("(o p) -> p o", p=P))
            bgX_sb = consts.tile([P, 1], F32, tag="bgX_sb")
            nc.gpsimd.dma_start(out=bgX_sb, in_=bgX.rearrange("(o p) -> p o", p=P))
            bgY_sb = consts.tile([P, 1], F32, tag="bgY_sb")
            nc.gpsimd.dma_start(out=bgY_sb, in_=bgY.rearrange("(o p) -> p o", p=P))
            # bg2 replicated to every partition (activation bias must be [P, 1])
            bg2r = consts.tile([P, 2], F32, tag="bg2r")
            nc.gpsimd.dma_start(out=bg2r, in_=bg2[None, :].partition_broadcast(P))
            # free-axis bias bv, replicated across partitions (bm shares the
            # slot later — disjoint lifetimes)
            bv_rep = brep.tile([P, D], F32, tag="brep")
            nc.gpsimd.dma_start(out=bv_rep, in_=bv[None, :].partition_broadcast(P))
        maskb = consts.tile([P, KJ], F32, tag="maskb")
        nc.vector.tensor_scalar_mul(maskb, mask_u8, NEG)

        WgX_f = consts.tile([P, DB], F32, tag="WgX_f")
        nc.sync.dma_start(out=WgX_f, in_=WgX[:, :])
        WgY_f = consts.tile([P, DB], F32, tag="WgY_f")
        nc.sync.dma_start(out=WgY_f, in_=WgY[:, :])
        WgX_sb = consts.tile([P, DB], adt, tag="WgX_sb")
        nc.gpsimd.tensor_copy(WgX_sb, WgX_f)
        WgY_sb = consts.tile([P, DB], adt, tag="WgY_sb")
        nc.gpsimd.tensor_copy(WgY_sb, WgY_f)
        # Wg2 columns replicated across 128 stationary columns: the z matmul
        # then emits each gate row already broadcast over all 128 partitions.
        Wg2_f = consts.tile([P, 2], F32, tag="Wg2_f")
        nc.sync.dma_start(out=Wg2_f, in_=Wg2[:, :])
        Wg2c = consts.tile([P, 2, P], adt, tag="Wg2c")
        nc.vector.tensor_copy(Wg2c, Wg2_f[:, :, None].to_broadcast((P, 2, P)))

        # ---- q/k projections, output transposed [d_out, s] ----
        def proj_T(xT, bias_sb, dstT, wb0, wb1):
            # Two j-tiles accumulate in flight per i-sweep so the first matmul
            # only needs W block i=0 (not the full 2MB half) — cuts the
            # startup dead time while Wq streams in.
            for half, wch in ((0, wb0), (1, wb1)):
                for sh in range(2):
                    sl = slice(sh * 512, (sh + 1) * 512)
                    for jp in (0, 2):
                        ps0 = psc.tile([P, 512], F32, tag="pacc", name="ps0")
                        ps1 = psc.tile([P, 512], F32, tag="pacc", name="ps1")
                        for i in range(NDT):
                            nc.tensor.matmul(
                                ps0,
                                wch[:, i, jp * P : (jp + 1) * P],
                                xT[:, i, sl],
                                start=(i == 0),
                                stop=(i == NDT - 1),
                            )
                            nc.tensor.matmul(
                                ps1,
                                wch[:, i, (jp + 1) * P : (jp + 2) * P],
                                xT[:, i, sl],
                                start=(i == 0),
                                stop=(i == NDT - 1),
                            )
                        j = half * 4 + jp  # d_out tile == head index
                        nc.vector.tensor_scalar_add(
                            dstT[:, j, sl], ps0, bias_sb[:, j : j + 1]
                        )
                        nc.vector.tensor_scalar_add(
                            dstT[:, j + 1, sl], ps1, bias_sb[:, j + 1 : j + 2]
                        )

        # ---- v projection, natural [s, d_out], + bv, into vh_aug ----
        def proj_v_tile(vT, wch0, wch1, m):
                ps = psc.tile([P, S], F32, tag="pacc")
                for half, wch in ((0, wch0), (1, wch1)):
                    sl = slice(half * 512, (half + 1) * 512)
                    for i in range(NDT):
                        nc.tensor.matmul(
                            ps[:, sl],
                            vT[:, i, m * P : (m + 1) * P],
                            wch[:, i, :],
                            start=(i == 0),
                            stop=(i == NDT - 1),
                        )
                nc.vector.tensor_tensor(
                    vh_aug[:, :, m, 0:DB],
                    ps.rearrange("p (h n) -> p h n", n=DB),
                    bv_rep.rearrange("p (h n) -> p h n", n=DB),
                    OP.add,
                )

        # Gate MLP split in two pipelined stages: gates_b(h) runs one v-tile
        # after gates_a(h), so its psz matmuls never stall the in-order PE
        # queue waiting on the DVE tt product.
        def gates_a(h):
            # gx = kh@WgX + bgX  (matmul on PE, bias-add on DVE)
            psx = psc.tile([P, S], F32, tag="pacc")
            for sh in range(2):
                sl = slice(sh * 512, (sh + 1) * 512)
                nc.tensor.matmul(
                    psx[:, sl], WgX_sb, khT[:, h, sl], start=True, stop=True
                )
            gx = gpool.tile([P, S], adt, tag="gx", bufs=1)
            nc.vector.tensor_scalar_add(gx, psx, bgX_sb)
            # tt = (qh@WgY + bgY) * gx  in one DVE scalar_tensor_tensor
            psy = psc.tile([P, S], F32, tag="pacc")
            for sh in range(2):
                sl = slice(sh * 512, (sh + 1) * 512)
                nc.tensor.matmul(
                    psy[:, sl], WgY_sb, qhT[:, h, sl], start=True, stop=True
                )
            tt = gpool.tile([P, S], adt, tag="tt")
            nc.vector.scalar_tensor_tensor(
                tt, psy, bgY_sb, gx, OP.add, OP.mult
            )
            return tt

        def gates_b(h, tt):
            # z matmuls with replicated Wg2 columns: every output partition
            # carries the same gate row -> no cross-partition broadcast needed.
            for gi, dstT in ((0, khT), (1, qhT)):
                psz = psc.tile([P, S], F32, tag="pacc")
                for sh in range(2):
                    sl = slice(sh * 512, (sh + 1) * 512)
                    nc.tensor.matmul(
                        psz[:, sl], Wg2c[:, gi, :], tt[:, sl], start=True, stop=True
                    )
                g = gpool.tile([P, S], adt, tag=f"g{gi}", bufs=1)
                nc.scalar.activation(
                    g, psz, AF.Sigmoid, bias=bg2r[:, gi : gi + 1]
                )
                nc.vector.tensor_tensor(dstT[:, h, :], dstT[:, h, :], g, OP.mult)

        # ---- attention helpers (chunked so score matmuls + exp interleave
        # with other PE work instead of serializing behind ACT) ----
        def new_PT():
            return big.tile([P, KJ, S], adt, tag="bigslab", name="PT")

        def sc(h, PT, kjs):
            # scores (transposed) + exp -> P^T rows [s_k-in-tile, kj, q]
            for kj in kjs:
                ps = psc.tile([P, S], F32, tag="pacc")
                for sh in range(2):
                    sl = slice(sh * 512, (sh + 1) * 512)
                    nc.tensor.matmul(
                        ps[:, sl],
                        khT[:, h, kj * P : (kj + 1) * P],
                        qhT[:, h, sl],
                        start=True,
                        stop=True,
                    )
                nc.scalar.activation(
                    PT[:, kj, :], ps, AF.Exp,
                    bias=maskb[:, kj : kj + 1], scale=SCALE,
                )

        def pv_half(h, PT, pt2, qis):
            # PV with fused denominator; normalize; transpose into A_T
            for qi in qis:
                pv = ppv.tile([P, DB + 1], F32, tag="pv")
                for kj in range(KJ):
                    nc.tensor.matmul(
                        pv,
                        PT[:, kj, qi * P : (qi + 1) * P],
                        vh_aug[:, h, kj, :],
                        start=(kj == 0),
                        stop=(kj == KJ - 1),
                    )
                rec = smalls.tile([P, 1], F32, tag="rec")
                nc.vector.reciprocal(rec, pv[:, DB : DB + 1])
                asb = attp.tile([P, P], pdt, tag="asb")
                nc.vector.tensor_scalar_mul(asb, pv[:, 0:DB], rec)
                nc.tensor.transpose(
                    pt2[:, qi * P : (qi + 1) * P], asb, identp
                )
            if qis[-1] == KJ - 1:
                nc.vector.tensor_copy(A_T[:, h, :], pt2)

        def pv_block(h, PT):
            pt2 = ptr.tile([P, NDT * P], pdt, tag="trps")
            pv_half(h, PT, pt2, [0, 1, 2, 3])
            pv_half(h, PT, pt2, [4, 5, 6, 7])

        # ---- main phase schedule ----
        proj_T(xTq, bq_sb, qhT, wq0, wq1)
        wk0, wk1 = load_w(Wk, nc.scalar)
        xTk = load_xT(k, nc.scalar)
        proj_T(xTk, bk_sb, khT, wk0, wk1)

        wv0, wv1 = load_w(Wv, nc.scalar)
        xTv = load_xT(v, nc.scalar)

        # v projection with the gate MLP interleaved per s-tile, and the
        # early heads' scores+exp chunks spread across the loop so ACT's exp
        # stream (the attention bottleneck at ~9us/head) starts ~2 heads
        # early without ever stalling the in-order PE queue. PT slabs 0/1
        # reuse the xTq/xTk big-pool slots (dead by then); PT2 takes slot 3.
        PTs = {}
        tts = {}
        for m in range(KJ):
            proj_v_tile(xTv, wv0, wv1, m)
            tts[m] = gates_a(m)
            if m >= 1:
                gates_b(m - 1, tts.pop(m - 1))
            if m == 2:
                PTs[0] = new_PT()
                sc(0, PTs[0], [0, 1])
            elif m == 3:
                sc(0, PTs[0], [2, 3])
            elif m == 4:
                sc(0, PTs[0], [4, 5])
            elif m == 5:
                sc(0, PTs[0], [6, 7])
            elif m == 6:
                PTs[1] = new_PT()
                sc(1, PTs[1], [0, 1, 2])
            elif m == 7:
                sc(1, PTs[1], [3, 4, 5])
        gates_b(KJ - 1, tts.pop(KJ - 1))
        sc(1, PTs[1], [6, 7])
        PTs[2] = new_PT()
        sc(2, PTs[2], [0, 1, 2, 3])

        # Wm + bm streamed during the attention loop (gpsimd casts overlap
        # the ACT-bound exp stream).
        bm_rep = brep.tile([P, D], F32, tag="brep")
        with nc.allow_non_contiguous_dma(reason="tiny partition-major loads"):
            nc.gpsimd.dma_start(out=bm_rep, in_=bm[None, :].partition_broadcast(P))
        wm0, wm1 = load_w(Wm, nc.gpsimd)

        sc(2, PTs[2], [4, 5, 6, 7])

        # Attention: exp of head h (ACT) interleaves with PV of h-3 (PE),
        # chunk by chunk; the PT ring is 4 deep (PT(h) takes PT(h-4)'s slot).
        for h in range(3, H):
            hp = h - 3
            PTs[h] = new_PT()
            pt2 = ptr.tile([P, NDT * P], pdt, tag="trps")
            sc(h, PTs[h], [0, 1, 2, 3])
            pv_half(hp, PTs[hp], pt2, [0, 1, 2, 3])
            sc(h, PTs[h], [4, 5, 6, 7])
            pv_half(hp, PTs[hp], pt2, [4, 5, 6, 7])
            PTs.pop(hp)
        for h in range(H - 3, H):
            pv_block(h, PTs.pop(h))

        # ---- merge: out = A @ Wm + bm (evicted + stored in column halves,
        # alternating output DMA across the two HWDGE queues) ----
        for m in range(KJ):
            ps = psc.tile([P, S], F32, tag="pacc")
            for half, wch in ((0, wm0), (1, wm1)):
                sl = slice(half * 512, (half + 1) * 512)
                for i in range(NDT):
                    nc.tensor.matmul(
                        ps[:, sl],
                        A_T[:, i, m * P : (m + 1) * P],
                        wch[:, i, :],
                        start=(i == 0),
                        stop=(i == NDT - 1),
                    )
                osb = outp.tile([P, 512], F32, tag="osb")
                nc.vector.tensor_tensor(osb, ps[:, sl], bm_rep[:, sl], OP.add)
                deng = nc.sync if half == 0 else nc.scalar
                deng.dma_start(
                    out=out[m * P : (m + 1) * P, half * 512 : (half + 1) * 512],
                    in_=osb,
                )

    nc.finalize()
    return nc


_NC_CACHE = {}


def _get_nc(key=("bf16", "bf16")):
    if key not in _NC_CACHE:
        _NC_CACHE[key] = build_nc(
            proj_bf16=(key[0] == "bf16"), attn_bf16=(key[1] == "bf16")
        )
    return _NC_CACHE[key]


def _f32(a):
    return np.ascontiguousarray(np.asarray(a, dtype=np.float32))


def kernel(v, k, q, mask, Wv, bv, Wk, bk, Wq, bq, Wm, bm,
           WgX, bgX, WgY, bgY, Wg2, bg2):
    from concourse.bass_utils import run_bass_kernel_spmd

    nc = _get_nc()
    nb = int(np.asarray(q).shape[0])
    shared = {
        "Wq": _f32(Wq), "Wk": _f32(Wk), "Wv": _f32(Wv), "Wm": _f32(Wm),
        "bq": _f32(bq), "bk": _f32(bk), "bv": _f32(bv), "bm": _f32(bm),
        "WgX": _f32(WgX), "WgY": _f32(WgY), "Wg2": _f32(Wg2),
        "bgX": _f32(bgX), "bgY": _f32(bgY), "bg2": _f32(bg2),
    }
    in_maps = []
    for b in range(nb):
        m = dict(shared)
        m["q"] = _f32(q[b])
        m["k"] = _f32(k[b])
        m["v"] = _f32(v[b])
        m["mask"] = np.ascontiguousarray(
            np.asarray(mask[b], dtype=np.bool_).reshape(S).view(np.uint8)
        )
        in_maps.append(m)
    res = run_bass_kernel_spmd(nc, in_maps, list(range(nb)))
    return np.stack([res.results[b]["out"] for b in range(nb)]).astype(np.float32)


# revision 27
# speedup vs baseline: 1.0226x; 1.0226x over previous
"""Trainium2 Bass kernel for gated multi-head attention (nn_MHAtt_41274635714591).

Strategy: data-parallel over batch — 8 batches onto 8 NeuronCores, one batch per
core, no collectives. Per core (S=1024, D=1024, H=8, DB=128):

  1. Inputs f32->bf16 cast on ACT — NOT gpsimd (3.2x slower; was the top
     bottleneck: PE idled ~25us at each input phase start waiting on casts).
     128x128 transposes on PE -> xT [d, s].
  2. Projections (bf16 matmuls, fp32 PSUM): qhT/khT = (x @ W + b)^T via
     lhsT=W-colblock, rhs=xT; vh in natural [s, d] layout straight into
     vh_aug whose extra all-ones column yields the softmax denominator
     for free from the PV matmul. Weights stream as 2MB column-halves on
     the gpsimd DMA queue (so they never head-of-line-block the x rows on
     the sync queue); casts: Wq on DVE (startup), Wk/Wv on ACT (slack in
     the q/k phases), Wm on gpsimd (during the ACT-bound attention phase).
  3. Gate MLP per head: gx = psx + bgX on DVE; tt = (psy + bgY) * gx in one
     DVE scalar_tensor_tensor; sigmoid on ACT. Gate rows are produced already
     broadcast across partitions by replicating the Wg2 column across the
     matmul's stationary dim; gates multiply khT/qhT in place.
  4. Scores computed TRANSPOSED: S^T[k,q] = lhsT=khT-chunk, rhs=qhT.
     exp(scale*x + maskbias_k) on ACT writes P^T directly — no P transposes.
     The mask folds in as a per-partition additive -1e9 bias. Heads 0/1
     pulled into the v phase (m=1,3) and head 2 right after, so ACT's exp
     stream (the attention-phase bottleneck at ~8.9us/head) starts ~3 heads
     early.
  5. PV: out[q, 0:129] = sum_k P^T-chunk^T @ vh_aug; column 128 is the
     denominator; normalize with DVE reciprocal + tensor_scalar.
  6. att tiles transposed on PE into A_T [d, s]; merge matmul with Wm
     col-halves streamed during the attention loop; + bm; DMA out.

The harness calls kernel(**full_inputs); we shard batch across cores with
run_bass_kernel_spmd and stack the per-core outputs.
"""

import math
import os
import sys

for _p in ("/opt/trn_rl_repo", "/root/.axon_site/_ro/trn_rl_repo"):
    if os.path.isdir(_p) and _p not in sys.path:
        sys.path.insert(0, _p)

import numpy as np

import concourse.bass as bass
import concourse.mybir as mybir
import concourse.tile as tile
from concourse import bacc
from concourse.masks import make_identity

F32 = mybir.dt.float32
BF16 = mybir.dt.bfloat16
U8 = mybir.dt.uint8
AF = mybir.ActivationFunctionType
OP = mybir.AluOpType

B, S, D, H = 8, 1024, 1024, 8
DB = D // H          # 128 per-head dim
P = 128              # partitions
KJ = S // P          # 8 tiles of 128 along s
NDT = D // P         # 8 tiles of 128 along d
SCALE = 1.0 / math.sqrt(DB)
NEG = -1e9


def build_nc(proj_bf16=True, attn_bf16=True, repeat=1):
    """Emit the per-core program. repeat>1 wraps the whole body in a
    device-side loop (for timing)."""
    assert proj_bf16 and attn_bf16
    pdt = BF16
    adt = BF16
    # Bacc (not plain Bass): its compile pipeline fuses multi-sem waits into
    # event semaphores — this container's walrus rejects instructions carrying
    # more than one sync wait — and inserts GPSIMD library / ACT table loads.
    nc = bacc.Bacc()

    q = nc.dram_tensor("q", [S, D], F32, kind="ExternalInput")
    k = nc.dram_tensor("k", [S, D], F32, kind="ExternalInput")
    v = nc.dram_tensor("v", [S, D], F32, kind="ExternalInput")
    mask = nc.dram_tensor("mask", [S], U8, kind="ExternalInput")
    Wq = nc.dram_tensor("Wq", [D, D], F32, kind="ExternalInput")
    Wk = nc.dram_tensor("Wk", [D, D], F32, kind="ExternalInput")
    Wv = nc.dram_tensor("Wv", [D, D], F32, kind="ExternalInput")
    Wm = nc.dram_tensor("Wm", [D, D], F32, kind="ExternalInput")
    bq = nc.dram_tensor("bq", [D], F32, kind="ExternalInput")
    bk = nc.dram_tensor("bk", [D], F32, kind="ExternalInput")
    bv = nc.dram_tensor("bv", [D], F32, kind="ExternalInput")
    bm = nc.dram_tensor("bm", [D], F32, kind="ExternalInput")
    WgX = nc.dram_tensor("WgX", [DB, DB], F32, kind="ExternalInput")
    WgY = nc.dram_tensor("WgY", [DB, DB], F32, kind="ExternalInput")
    Wg2 = nc.dram_tensor("Wg2", [DB, 2], F32, kind="ExternalInput")
    bgX = nc.dram_tensor("bgX", [DB], F32, kind="ExternalInput")
    bgY = nc.dram_tensor("bgY", [DB], F32, kind="ExternalInput")
    bg2 = nc.dram_tensor("bg2", [2], F32, kind="ExternalInput")
    out = nc.dram_tensor("out", [S, D], F32, kind="ExternalOutput")

    from contextlib import ExitStack

    with tile.TileContext(nc) as tc, ExitStack() as ctx:
        consts = ctx.enter_context(tc.tile_pool(name="consts", bufs=1))
        persist = ctx.enter_context(tc.tile_pool(name="persist", bufs=1))
        big = ctx.enter_context(tc.tile_pool(name="big", bufs=4))
        xrow = ctx.enter_context(tc.tile_pool(name="xrow", bufs=5))
        xbrow = ctx.enter_context(tc.tile_pool(name="xbrow", bufs=2))
        wstream = ctx.enter_context(tc.tile_pool(name="wstream", bufs=3))
        wconv = ctx.enter_context(tc.tile_pool(name="wconv", bufs=2))
        gpool = ctx.enter_context(tc.tile_pool(name="gpool", bufs=2))
        attp = ctx.enter_context(tc.tile_pool(name="attp", bufs=2))
        smalls = ctx.enter_context(tc.tile_pool(name="smalls", bufs=2))
        outp = ctx.enter_context(tc.tile_pool(name="outp", bufs=2))
        brep = ctx.enter_context(tc.tile_pool(name="brep", bufs=1))
        # PSUM: psc 2x[128,1024]f32 (4 banks) + ppv 2x[128,129]f32 (2 banks)
        # + ptr 2x[128,1024]bf16 (2 banks) = 8 banks
        psc = ctx.enter_context(tc.tile_pool(name="psc", bufs=2, space="PSUM"))
        ppv = ctx.enter_context(tc.tile_pool(name="ppv", bufs=2, space="PSUM"))
        ptr = ctx.enter_context(tc.tile_pool(name="ptr", bufs=2, space="PSUM"))

        # ---- identity + persistent activations (identity + the vh_aug ones
        # column are input-independent — emitted before the timing loop) ----
        identp = consts.tile([P, P], pdt, tag="identp")
        make_identity(nc, identp)

        qhT = persist.tile([P, H, S], adt, tag="qhT")   # [db, h, s] = (q@Wq+b)^T
        khT = persist.tile([P, H, S], adt, tag="khT")
        vh_aug = persist.tile([P, H, KJ, DB + 1], adt, tag="vh_aug")
        nc.vector.memset(vh_aug[:, :, :, DB : DB + 1], 1.0)
        A_T = persist.tile([P, H, S], pdt, tag="A_T")   # attention out, transposed

        if repeat > 1:
            ctx.enter_context(tc.For_i(0, repeat, 1))

        def cast(eng, dst, src):
            if eng is nc.scalar:
                nc.scalar.copy(dst, src)
            else:
                eng.tensor_copy(dst, src)

        # ---- input transpose: x [s, d] -> xT [d-in-tile, i, s] (bf16) ----
        # Each 128-row block loads as TWO half-row DMAs fired simultaneously
        # on the sync and scalar HWDGE queues (per-queue descriptor
        # generation is the DMA latency driver, so splitting halves the
        # arrival time), one block ahead of the cast/transpose consumption.
        def load_xT(xdram, ceng):
            xT = big.tile([P, NDT, S], pdt, tag="bigslab")
            xfs = {}

            def issue(m):
                if m >= KJ:
                    return
                hs = []
                for half, deng in ((0, nc.sync), (1, nc.scalar)):
                    xf = xrow.tile([P, 512], F32, tag="xrow")
                    deng.dma_start(
                        out=xf,
                        in_=xdram[m * P : (m + 1) * P,
                                  half * 512 : (half + 1) * 512],
                    )
                    hs.append(xf)
                xfs[m] = hs

            issue(0)
            issue(1)
            for m in range(KJ):
                xb = xbrow.tile([P, D], pdt, tag="xbrow")
                halves = xfs.pop(m)
                pt = ptr.tile([P, NDT * P], pdt, tag="trps")
                for half in range(2):
                    sl = slice(half * 512, (half + 1) * 512)
                    cast(ceng, xb[:, sl], halves[half])
                    for dj in range(half * 4, half * 4 + 4):
                        nc.tensor.transpose(
                            pt[:, dj * P : (dj + 1) * P],
                            xb[:, dj * P : (dj + 1) * P],
                            identp,
                        )
                issue(m + 2)
                nc.vector.tensor_copy(
                    xT[:, :, m * P : (m + 1) * P],
                    pt.rearrange("p (a b) -> p a b", b=P),
                )
            return xT

        def load_w(Wdram, ceng):
            """Stream W in column-half order ([D, 512] then [D, 512]) as 2KB
            descriptor chunks on the gpsimd SWDGE queue: the projections'
            first accumulation sweep only needs half-0 chunk i=0, and half-1
            streams while half-0 is being consumed."""
            wbs = []
            for half in range(2):
                wb = wconv.tile(
                    [P, NDT, 512], pdt, tag=f"wchb{half}", name="wb"
                )
                wsrc = Wdram[:, half * 512 : (half + 1) * 512].rearrange(
                    "(i p) n -> p i n", p=P
                )
                for c in range(0, NDT, 2):
                    wf = wstream.tile([P, 2, 512], F32, tag="wch")
                    nc.gpsimd.dma_start(out=wf, in_=wsrc[:, c : c + 2, :])
                    cast(ceng, wb[:, c : c + 2, :], wf)
                wbs.append(wb)
            return wbs[0], wbs[1]

        # ---- startup: critical DMAs first (Wq chunks on the gpsimd queue,
        # q half-rows on the sync+scalar queues — all stream concurrently) ----
        wq0, wq1 = load_w(Wq, nc.vector)
        xTq = load_xT(q, nc.scalar)

        # ---- constants / small prep (nothing here is needed before ~15us;
        # emitted after the startup DMAs so the tiny SWDGE loads don't delay
        # them in the queues) ----
        with nc.allow_non_contiguous_dma(reason="tiny partition-major loads"):
            mask_u8 = consts.tile([P, KJ], U8, tag="mask_u8")
            nc.gpsimd.dma_start(
                out=mask_u8, in_=mask.rearrange("(o p) -> p o", p=P)
            )
            bq_sb = consts.tile([P, NDT], F32, tag="bq_sb")
            nc.gpsimd.dma_start(out=bq_sb, in_=bq.rearrange("(o p) -> p o", p=P))
            bk_sb = consts.tile([P, NDT], F32, tag="bk_sb")
            nc.gpsimd.dma_start(out=bk_sb, in_=bk.rearrange("(o p) -> p o", p=P))
            bgX_sb = consts.tile([P, 1], F32, tag="bgX_sb")
            nc.gpsimd.dma_start(out=bgX_sb, in_=bgX.rearrange("(o p) -> p o", p=P))
            bgY_sb = consts.tile([P, 1], F32, tag="bgY_sb")
            nc.gpsimd.dma_start(out=bgY_sb, in_=bgY.rearrange("(o p) -> p o", p=P))
            # bg2 replicated to every partition (activation bias must be [P, 1])
            bg2r = consts.tile([P, 2], F32, tag="bg2r")
            nc.gpsimd.dma_start(out=bg2r, in_=bg2[None, :].partition_broadcast(P))
            # free-axis bias bv, replicated across partitions (bm shares the
            # slot later — disjoint lifetimes)
            bv_rep = brep.tile([P, D], F32, tag="brep")
            nc.gpsimd.dma_start(out=bv_rep, in_=bv[None, :].partition_broadcast(P))
        maskb = consts.tile([P, KJ], F32, tag="maskb")
        nc.vector.tensor_scalar_mul(maskb, mask_u8, NEG)

        WgX_f = consts.tile([P, DB], F32, tag="WgX_f")
        nc.sync.dma_start(out=WgX_f, in_=WgX[:, :])
        WgY_f = consts.tile([P, DB], F32, tag="WgY_f")
        nc.sync.dma_start(out=WgY_f, in_=WgY[:, :])
        WgX_sb = consts.tile([P, DB], adt, tag="WgX_sb")
        nc.gpsimd.tensor_copy(WgX_sb, WgX_f)
        WgY_sb = consts.tile([P, DB], adt, tag="WgY_sb")
        nc.gpsimd.tensor_copy(WgY_sb, WgY_f)
        # Wg2 columns replicated across 128 stationary columns: the z matmul
        # then emits each gate row already broadcast over all 128 partitions.
        Wg2_f = consts.tile([P, 2], F32, tag="Wg2_f")
        nc.sync.dma_start(out=Wg2_f, in_=Wg2[:, :])
        Wg2c = consts.tile([P, 2, P], adt, tag="Wg2c")
        nc.vector.tensor_copy(Wg2c, Wg2_f[:, :, None].to_broadcast((P, 2, P)))

        # ---- q/k projections, output transposed [d_out, s] ----
        def proj_T(xT, bias_sb, dstT, wb0, wb1):
            # Two j-tiles accumulate in flight per i-sweep so the first matmul
            # only needs W block i=0 (not the full 2MB half) — cuts the
            # startup dead time while Wq streams in.
            for half, wch in ((0, wb0), (1, wb1)):
                for sh in range(2):
                    sl = slice(sh * 512, (sh + 1) * 512)
                    for jp in (0, 2):
                        ps0 = psc.tile([P, 512], F32, tag="pacc", name="ps0")
                        ps1 = psc.tile([P, 512], F32, tag="pacc", name="ps1")
                        for i in range(NDT):
                            nc.tensor.matmul(
                                ps0,
                                wch[:, i, jp * P : (jp + 1) * P],
                                xT[:, i, sl],
                                start=(i == 0),
                                stop=(i == NDT - 1),
                            )
                            nc.tensor.matmul(
                                ps1,
                                wch[:, i, (jp + 1) * P : (jp + 2) * P],
                                xT[:, i, sl],
                                start=(i == 0),
                                stop=(i == NDT - 1),
                            )
                        j = half * 4 + jp  # d_out tile == head index
                        nc.vector.tensor_scalar_add(
                            dstT[:, j, sl], ps0, bias_sb[:, j : j + 1]
                        )
                        nc.vector.tensor_scalar_add(
                            dstT[:, j + 1, sl], ps1, bias_sb[:, j + 1 : j + 2]
                        )

        # ---- v projection, natural [s, d_out], + bv, into vh_aug ----
        def proj_v_tile(vT, wch0, wch1, m):
                ps = psc.tile([P, S], F32, tag="pacc")
                for half, wch in ((0, wch0), (1, wch1)):
                    sl = slice(half * 512, (half + 1) * 512)
                    for i in range(NDT):
                        nc.tensor.matmul(
                            ps[:, sl],
                            vT[:, i, m * P : (m + 1) * P],
                            wch[:, i, :],
                            start=(i == 0),
                            stop=(i == NDT - 1),
                        )
                nc.vector.tensor_tensor(
                    vh_aug[:, :, m, 0:DB],
                    ps.rearrange("p (h n) -> p h n", n=DB),
                    bv_rep.rearrange("p (h n) -> p h n", n=DB),
                    OP.add,
                )

        # Gate MLP split in two pipelined stages: gates_b(h) runs one v-tile
        # after gates_a(h), so its psz matmuls never stall the in-order PE
        # queue waiting on the DVE tt product.
        def gates_a(h):
            # gx = kh@WgX + bgX  (matmul on PE, bias-add on DVE)
            psx = psc.tile([P, S], F32, tag="pacc")
            for sh in range(2):
                sl = slice(sh * 512, (sh + 1) * 512)
                nc.tensor.matmul(
                    psx[:, sl], WgX_sb, khT[:, h, sl], start=True, stop=True
                )
            gx = gpool.tile([P, S], adt, tag="gx", bufs=1)
            nc.vector.tensor_scalar_add(gx, psx, bgX_sb)
            # tt = (qh@WgY + bgY) * gx  in one DVE scalar_tensor_tensor
            psy = psc.tile([P, S], F32, tag="pacc")
            for sh in range(2):
                sl = slice(sh * 512, (sh + 1) * 512)
                nc.tensor.matmul(
                    psy[:, sl], WgY_sb, qhT[:, h, sl], start=True, stop=True
                )
            tt = gpool.tile([P, S], adt, tag="tt")
            nc.vector.scalar_tensor_tensor(
                tt, psy, bgY_sb, gx, OP.add, OP.mult
            )
            return tt

        def gates_b(h, tt):
            # z matmuls with replicated Wg2 columns: every output partition
            # carries the same gate row -> no cross-partition broadcast needed.
            for gi, dstT in ((0, khT), (1, qhT)):
                psz = psc.tile([P, S], F32, tag="pacc")
                for sh in range(2):
                    sl = slice(sh * 512, (sh + 1) * 512)
                    nc.tensor.matmul(
                        psz[:, sl], Wg2c[:, gi, :], tt[:, sl], start=True, stop=True
                    )
                g = gpool.tile([P, S], adt, tag=f"g{gi}", bufs=1)
                nc.scalar.activation(
                    g, psz, AF.Sigmoid, bias=bg2r[:, gi : gi + 1]
                )
                nc.vector.tensor_tensor(dstT[:, h, :], dstT[:, h, :], g, OP.mult)

        # ---- attention helpers (chunked so score matmuls + exp interleave
        # with other PE work instead of serializing behind ACT) ----
        def new_PT():
            return big.tile([P, KJ, S], adt, tag="bigslab", name="PT")

        def sc(h, PT, kjs):
            # scores (transposed) + exp -> P^T rows [s_k-in-tile, kj, q]
            for kj in kjs:
                ps = psc.tile([P, S], F32, tag="pacc")
                for sh in range(2):
                    sl = slice(sh * 512, (sh + 1) * 512)
                    nc.tensor.matmul(
                        ps[:, sl],
                        khT[:, h, kj * P : (kj + 1) * P],
                        qhT[:, h, sl],
                        start=True,
                        stop=True,
                    )
                nc.scalar.activation(
                    PT[:, kj, :], ps, AF.Exp,
                    bias=maskb[:, kj : kj + 1], scale=SCALE,
                )

        def pv_half(h, PT, pt2, qis):
            # PV with fused denominator; normalize; transpose into A_T
            for qi in qis:
                pv = ppv.tile([P, DB + 1], F32, tag="pv")
                for kj in range(KJ):
                    nc.tensor.matmul(
                        pv,
                        PT[:, kj, qi * P : (qi + 1) * P],
                        vh_aug[:, h, kj, :],
                        start=(kj == 0),
                        stop=(kj == KJ - 1),
                    )
                rec = smalls.tile([P, 1], F32, tag="rec")
                nc.vector.reciprocal(rec, pv[:, DB : DB + 1])
                asb = attp.tile([P, P], pdt, tag="asb")
                nc.vector.tensor_scalar_mul(asb, pv[:, 0:DB], rec)
                nc.tensor.transpose(
                    pt2[:, qi * P : (qi + 1) * P], asb, identp
                )
            if qis[-1] == KJ - 1:
                nc.vector.tensor_copy(A_T[:, h, :], pt2)

        def pv_block(h, PT):
            pt2 = ptr.tile([P, NDT * P], pdt, tag="trps")
            pv_half(h, PT, pt2, [0, 1, 2, 3])
            pv_half(h, PT, pt2, [4, 5, 6, 7])

        # ---- main phase schedule ----
        proj_T(xTq, bq_sb, qhT, wq0, wq1)
        wk0, wk1 = load_w(Wk, nc.scalar)
        xTk = load_xT(k, nc.scalar)
        proj_T(xTk, bk_sb, khT, wk0, wk1)

        wv0, wv1 = load_w(Wv, nc.scalar)
        xTv = load_xT(v, nc.scalar)

        # v projection with the gate MLP interleaved per s-tile, and the
        # early heads' scores+exp chunks spread across the loop so ACT's exp
        # stream (the attention bottleneck at ~9us/head) starts ~2 heads
        # early without ever stalling the in-order PE queue. PT slabs 0/1
        # reuse the xTq/xTk big-pool slots (dead by then); PT2 takes slot 3.
        PTs = {}
        tts = {}
        for m in range(KJ):
            proj_v_tile(xTv, wv0, wv1, m)
            tts[m] = gates_a(m)
            if m >= 1:
                gates_b(m - 1, tts.pop(m - 1))
            if m == 2:
                PTs[0] = new_PT()
                sc(0, PTs[0], [0, 1])
            elif m == 3:
                sc(0, PTs[0], [2, 3])
            elif m == 4:
                sc(0, PTs[0], [4, 5])
            elif m == 5:
                sc(0, PTs[0], [6, 7])
            elif m == 6:
                PTs[1] = new_PT()
                sc(1, PTs[1], [0, 1, 2])
            elif m == 7:
                sc(1, PTs[1], [3, 4, 5])
        gates_b(KJ - 1, tts.pop(KJ - 1))
        sc(1, PTs[1], [6, 7])
        PTs[2] = new_PT()
        sc(2, PTs[2], [0, 1, 2, 3])

        # Wm + bm streamed during the attention loop (gpsimd casts overlap
        # the ACT-bound exp stream).
        bm_rep = brep.tile([P, D], F32, tag="brep")
        with nc.allow_non_contiguous_dma(reason="tiny partition-major loads"):
            nc.gpsimd.dma_start(out=bm_rep, in_=bm[None, :].partition_broadcast(P))
        wm0, wm1 = load_w(Wm, nc.gpsimd)

        sc(2, PTs[2], [4, 5, 6, 7])

        # Attention: exp of head h (ACT) interleaves with PV of h-3 (PE),
        # chunk by chunk; the PT ring is 4 deep (PT(h) takes PT(h-4)'s slot).
        for h in range(3, H):
            hp = h - 3
            PTs[h] = new_PT()
            pt2 = ptr.tile([P, NDT * P], pdt, tag="trps")
            sc(h, PTs[h], [0, 1, 2, 3])
            pv_half(hp, PTs[hp], pt2, [0, 1, 2, 3])
            sc(h, PTs[h], [4, 5, 6, 7])
            pv_half(hp, PTs[hp], pt2, [4, 5, 6, 7])
            PTs.pop(hp)
        for h in range(H - 3, H):
            pv_block(h, PTs.pop(h))

        # ---- merge: out = A @ Wm + bm (evicted + stored in column halves,
        # alternating output DMA across the two HWDGE queues) ----
        for m in range(KJ):
            ps = psc.tile([P, S], F32, tag="pacc")
            for half, wch in ((0, wm0), (1, wm1)):
                sl = slice(half * 512, (half + 1) * 512)
                for i in range(NDT):
                    nc.tensor.matmul(
                        ps[:, sl],
                        A_T[:, i, m * P : (m + 1) * P],
                        wch[:, i, :],
                        start=(i == 0),
                        stop=(i == NDT - 1),
                    )
                osb = outp.tile([P, 512], F32, tag="osb")
                nc.vector.tensor_tensor(osb, ps[:, sl], bm_rep[:, sl], OP.add)
                deng = nc.sync if half == 0 else nc.scalar
                deng.dma_start(
                    out=out[m * P : (m + 1) * P, half * 512 : (half + 1) * 512],
                    in_=osb,
                )

    nc.finalize()
    return nc


_NC_CACHE = {}


def _get_nc(key=("bf16", "bf16")):
    if key not in _NC_CACHE:
        _NC_CACHE[key] = build_nc(
            proj_bf16=(key[0] == "bf16"), attn_bf16=(key[1] == "bf16")
        )
    return _NC_CACHE[key]


def _f32(a):
    return np.ascontiguousarray(np.asarray(a, dtype=np.float32))


def kernel(v, k, q, mask, Wv, bv, Wk, bk, Wq, bq, Wm, bm,
           WgX, bgX, WgY, bgY, Wg2, bg2):
    from concourse.bass_utils import run_bass_kernel_spmd

    nc = _get_nc()
    nb = int(np.asarray(q).shape[0])
    shared = {
        "Wq": _f32(Wq), "Wk": _f32(Wk), "Wv": _f32(Wv), "Wm": _f32(Wm),
        "bq": _f32(bq), "bk": _f32(bk), "bv": _f32(bv), "bm": _f32(bm),
        "WgX": _f32(WgX), "WgY": _f32(WgY), "Wg2": _f32(Wg2),
        "bgX": _f32(bgX), "bgY": _f32(bgY), "bg2": _f32(bg2),
    }
    in_maps = []
    for b in range(nb):
        m = dict(shared)
        m["q"] = _f32(q[b])
        m["k"] = _f32(k[b])
        m["v"] = _f32(v[b])
        m["mask"] = np.ascontiguousarray(
            np.asarray(mask[b], dtype=np.bool_).reshape(S).view(np.uint8)
        )
        in_maps.append(m)
    res = run_bass_kernel_spmd(nc, in_maps, list(range(nb)))
    return np.stack([res.results[b]["out"] for b in range(nb)]).astype(np.float32)


# revision 28
# speedup vs baseline: 1.0284x; 1.0056x over previous
"""Trainium2 Bass kernel for gated multi-head attention (nn_MHAtt_41274635714591).

Strategy: data-parallel over batch — 8 batches onto 8 NeuronCores, one batch per
core, no collectives. Per core (S=1024, D=1024, H=8, DB=128):

  1. Inputs f32->bf16 cast on ACT — NOT gpsimd (3.2x slower; was the top
     bottleneck: PE idled ~25us at each input phase start waiting on casts).
     128x128 transposes on PE -> xT [d, s].
  2. Projections (bf16 matmuls, fp32 PSUM): qhT/khT = (x @ W + b)^T via
     lhsT=W-colblock, rhs=xT; vh in natural [s, d] layout straight into
     vh_aug whose extra all-ones column yields the softmax denominator
     for free from the PV matmul. Weights stream as 2MB column-halves on
     the gpsimd DMA queue (so they never head-of-line-block the x rows on
     the sync queue); casts: Wq on DVE (startup), Wk/Wv on ACT (slack in
     the q/k phases), Wm on gpsimd (during the ACT-bound attention phase).
  3. Gate MLP per head: gx = psx + bgX on DVE; tt = (psy + bgY) * gx in one
     DVE scalar_tensor_tensor; sigmoid on ACT. Gate rows are produced already
     broadcast across partitions by replicating the Wg2 column across the
     matmul's stationary dim; gates multiply khT/qhT in place.
  4. Scores computed TRANSPOSED: S^T[k,q] = lhsT=khT-chunk, rhs=qhT.
     exp(scale*x + maskbias_k) on ACT writes P^T directly — no P transposes.
     The mask folds in as a per-partition additive -1e9 bias. Heads 0/1
     pulled into the v phase (m=1,3) and head 2 right after, so ACT's exp
     stream (the attention-phase bottleneck at ~8.9us/head) starts ~3 heads
     early.
  5. PV: out[q, 0:129] = sum_k P^T-chunk^T @ vh_aug; column 128 is the
     denominator; normalize with DVE reciprocal + tensor_scalar.
  6. att tiles transposed on PE into A_T [d, s]; merge matmul with Wm
     col-halves streamed during the attention loop; + bm; DMA out.

The harness calls kernel(**full_inputs); we shard batch across cores with
run_bass_kernel_spmd and stack the per-core outputs.
"""

import math
import os
import sys

for _p in ("/opt/trn_rl_repo", "/root/.axon_site/_ro/trn_rl_repo"):
    if os.path.isdir(_p) and _p not in sys.path:
        sys.path.insert(0, _p)

import numpy as np

import concourse.bass as bass
import concourse.mybir as mybir
import concourse.tile as tile
from concourse import bacc
from concourse.masks import make_identity

F32 = mybir.dt.float32
BF16 = mybir.dt.bfloat16
U8 = mybir.dt.uint8
AF = mybir.ActivationFunctionType
OP = mybir.AluOpType

B, S, D, H = 8, 1024, 1024, 8
DB = D // H          # 128 per-head dim
P = 128              # partitions
KJ = S // P          # 8 tiles of 128 along s
NDT = D // P         # 8 tiles of 128 along d
SCALE = 1.0 / math.sqrt(DB)
NEG = -1e9


def build_nc(proj_bf16=True, attn_bf16=True, repeat=1):
    """Emit the per-core program. repeat>1 wraps the whole body in a
    device-side loop (for timing)."""
    assert proj_bf16 and attn_bf16
    pdt = BF16
    adt = BF16
    # Bacc (not plain Bass): its compile pipeline fuses multi-sem waits into
    # event semaphores — this container's walrus rejects instructions carrying
    # more than one sync wait — and inserts GPSIMD library / ACT table loads.
    nc = bacc.Bacc()

    q = nc.dram_tensor("q", [S, D], F32, kind="ExternalInput")
    k = nc.dram_tensor("k", [S, D], F32, kind="ExternalInput")
    v = nc.dram_tensor("v", [S, D], F32, kind="ExternalInput")
    mask = nc.dram_tensor("mask", [S], U8, kind="ExternalInput")
    Wq = nc.dram_tensor("Wq", [D, D], F32, kind="ExternalInput")
    Wk = nc.dram_tensor("Wk", [D, D], F32, kind="ExternalInput")
    Wv = nc.dram_tensor("Wv", [D, D], F32, kind="ExternalInput")
    Wm = nc.dram_tensor("Wm", [D, D], F32, kind="ExternalInput")
    bq = nc.dram_tensor("bq", [D], F32, kind="ExternalInput")
    bk = nc.dram_tensor("bk", [D], F32, kind="ExternalInput")
    bv = nc.dram_tensor("bv", [D], F32, kind="ExternalInput")
    bm = nc.dram_tensor("bm", [D], F32, kind="ExternalInput")
    WgX = nc.dram_tensor("WgX", [DB, DB], F32, kind="ExternalInput")
    WgY = nc.dram_tensor("WgY", [DB, DB], F32, kind="ExternalInput")
    Wg2 = nc.dram_tensor("Wg2", [DB, 2], F32, kind="ExternalInput")
    bgX = nc.dram_tensor("bgX", [DB], F32, kind="ExternalInput")
    bgY = nc.dram_tensor("bgY", [DB], F32, kind="ExternalInput")
    bg2 = nc.dram_tensor("bg2", [2], F32, kind="ExternalInput")
    out = nc.dram_tensor("out", [S, D], F32, kind="ExternalOutput")

    from contextlib import ExitStack

    with tile.TileContext(nc) as tc, ExitStack() as ctx:
        consts = ctx.enter_context(tc.tile_pool(name="consts", bufs=1))
        persist = ctx.enter_context(tc.tile_pool(name="persist", bufs=1))
        big = ctx.enter_context(tc.tile_pool(name="big", bufs=4))
        xrow = ctx.enter_context(tc.tile_pool(name="xrow", bufs=5))
        xbrow = ctx.enter_context(tc.tile_pool(name="xbrow", bufs=2))
        wstream = ctx.enter_context(tc.tile_pool(name="wstream", bufs=3))
        wconv = ctx.enter_context(tc.tile_pool(name="wconv", bufs=2))
        gpool = ctx.enter_context(tc.tile_pool(name="gpool", bufs=2))
        attp = ctx.enter_context(tc.tile_pool(name="attp", bufs=2))
        smalls = ctx.enter_context(tc.tile_pool(name="smalls", bufs=2))
        outp = ctx.enter_context(tc.tile_pool(name="outp", bufs=2))
        brep = ctx.enter_context(tc.tile_pool(name="brep", bufs=1))
        # PSUM: psc 2x[128,1024]f32 (4 banks) + ppv 2x[128,129]f32 (2 banks)
        # + ptr 2x[128,1024]bf16 (2 banks) = 8 banks
        psc = ctx.enter_context(tc.tile_pool(name="psc", bufs=2, space="PSUM"))
        ppv = ctx.enter_context(tc.tile_pool(name="ppv", bufs=2, space="PSUM"))
        ptr = ctx.enter_context(tc.tile_pool(name="ptr", bufs=2, space="PSUM"))

        # ---- identity + persistent activations (identity + the vh_aug ones
        # column are input-independent — emitted before the timing loop) ----
        identp = consts.tile([P, P], pdt, tag="identp")
        make_identity(nc, identp)

        qhT = persist.tile([P, H, S], adt, tag="qhT")   # [db, h, s] = (q@Wq+b)^T
        khT = persist.tile([P, H, S], adt, tag="khT")
        vh_aug = persist.tile([P, H, KJ, DB + 1], adt, tag="vh_aug")
        nc.vector.memset(vh_aug[:, :, :, DB : DB + 1], 1.0)
        A_T = persist.tile([P, H, S], pdt, tag="A_T")   # attention out, transposed

        if repeat > 1:
            ctx.enter_context(tc.For_i(0, repeat, 1))

        def cast(eng, dst, src):
            if eng is nc.scalar:
                nc.scalar.copy(dst, src)
            else:
                eng.tensor_copy(dst, src)

        # ---- input transpose: x [s, d] -> xT [d-in-tile, i, s] (bf16) ----
        # Each 128-row block loads as TWO half-row DMAs fired simultaneously
        # on the sync and scalar HWDGE queues (per-queue descriptor
        # generation is the DMA latency driver, so splitting halves the
        # arrival time), one block ahead of the cast/transpose consumption.
        def load_xT(xdram, ceng):
            xT = big.tile([P, NDT, S], pdt, tag="bigslab")
            xfs = {}

            def issue(m):
                if m >= KJ:
                    return
                hs = []
                for half, deng in ((0, nc.sync), (1, nc.scalar)):
                    xf = xrow.tile([P, 512], F32, tag="xrow")
                    deng.dma_start(
                        out=xf,
                        in_=xdram[m * P : (m + 1) * P,
                                  half * 512 : (half + 1) * 512],
                    )
                    hs.append(xf)
                xfs[m] = hs

            issue(0)
            issue(1)
            for m in range(KJ):
                xb = xbrow.tile([P, D], pdt, tag="xbrow")
                halves = xfs.pop(m)
                pt = ptr.tile([P, NDT * P], pdt, tag="trps")
                # the two half-casts run on DIFFERENT engines concurrently,
                # halving the cast latency the transposes wait on
                for half in range(2):
                    sl = slice(half * 512, (half + 1) * 512)
                    cast(ceng if half == 0 else nc.vector,
                         xb[:, sl], halves[half])
                for half in range(2):
                    for dj in range(half * 4, half * 4 + 4):
                        nc.tensor.transpose(
                            pt[:, dj * P : (dj + 1) * P],
                            xb[:, dj * P : (dj + 1) * P],
                            identp,
                        )
                issue(m + 2)
                nc.vector.tensor_copy(
                    xT[:, :, m * P : (m + 1) * P],
                    pt.rearrange("p (a b) -> p a b", b=P),
                )
            return xT

        def load_w(Wdram, ceng):
            """Stream W in column-half order ([D, 512] then [D, 512]) as 2KB
            descriptor chunks on the gpsimd SWDGE queue: the projections'
            first accumulation sweep only needs half-0 chunk i=0, and half-1
            streams while half-0 is being consumed."""
            wbs = []
            for half in range(2):
                wb = wconv.tile(
                    [P, NDT, 512], pdt, tag=f"wchb{half}", name="wb"
                )
                wsrc = Wdram[:, half * 512 : (half + 1) * 512].rearrange(
                    "(i p) n -> p i n", p=P
                )
                for c in range(0, NDT, 2):
                    wf = wstream.tile([P, 2, 512], F32, tag="wch")
                    nc.gpsimd.dma_start(out=wf, in_=wsrc[:, c : c + 2, :])
                    cast(ceng, wb[:, c : c + 2, :], wf)
                wbs.append(wb)
            return wbs[0], wbs[1]

        # ---- startup: critical DMAs first (Wq chunks on the gpsimd queue,
        # q half-rows on the sync+scalar queues — all stream concurrently) ----
        wq0, wq1 = load_w(Wq, nc.vector)
        xTq = load_xT(q, nc.scalar)

        # ---- constants / small prep (nothing here is needed before ~15us;
        # emitted after the startup DMAs so the tiny SWDGE loads don't delay
        # them in the queues) ----
        with nc.allow_non_contiguous_dma(reason="tiny partition-major loads"):
            mask_u8 = consts.tile([P, KJ], U8, tag="mask_u8")
            nc.gpsimd.dma_start(
                out=mask_u8, in_=mask.rearrange("(o p) -> p o", p=P)
            )
            bq_sb = consts.tile([P, NDT], F32, tag="bq_sb")
            nc.gpsimd.dma_start(out=bq_sb, in_=bq.rearrange("(o p) -> p o", p=P))
            bk_sb = consts.tile([P, NDT], F32, tag="bk_sb")
            nc.gpsimd.dma_start(out=bk_sb, in_=bk.rearrange("(o p) -> p o", p=P))
            bgX_sb = consts.tile([P, 1], F32, tag="bgX_sb")
            nc.gpsimd.dma_start(out=bgX_sb, in_=bgX.rearrange("(o p) -> p o", p=P))
            bgY_sb = consts.tile([P, 1], F32, tag="bgY_sb")
            nc.gpsimd.dma_start(out=bgY_sb, in_=bgY.rearrange("(o p) -> p o", p=P))
            # bg2 replicated to every partition (activation bias must be [P, 1])
            bg2r = consts.tile([P, 2], F32, tag="bg2r")
            nc.gpsimd.dma_start(out=bg2r, in_=bg2[None, :].partition_broadcast(P))
            # free-axis bias bv, replicated across partitions (bm shares the
            # slot later — disjoint lifetimes)
            bv_rep = brep.tile([P, D], F32, tag="brep")
            nc.gpsimd.dma_start(out=bv_rep, in_=bv[None, :].partition_broadcast(P))
        maskb = consts.tile([P, KJ], F32, tag="maskb")
        nc.vector.tensor_scalar_mul(maskb, mask_u8, NEG)

        WgX_f = consts.tile([P, DB], F32, tag="WgX_f")
        nc.sync.dma_start(out=WgX_f, in_=WgX[:, :])
        WgY_f = consts.tile([P, DB], F32, tag="WgY_f")
        nc.sync.dma_start(out=WgY_f, in_=WgY[:, :])
        WgX_sb = consts.tile([P, DB], adt, tag="WgX_sb")
        nc.gpsimd.tensor_copy(WgX_sb, WgX_f)
        WgY_sb = consts.tile([P, DB], adt, tag="WgY_sb")
        nc.gpsimd.tensor_copy(WgY_sb, WgY_f)
        # Wg2 columns replicated across 128 stationary columns: the z matmul
        # then emits each gate row already broadcast over all 128 partitions.
        Wg2_f = consts.tile([P, 2], F32, tag="Wg2_f")
        nc.sync.dma_start(out=Wg2_f, in_=Wg2[:, :])
        Wg2c = consts.tile([P, 2, P], adt, tag="Wg2c")
        nc.vector.tensor_copy(Wg2c, Wg2_f[:, :, None].to_broadcast((P, 2, P)))

        # ---- q/k projections, output transposed [d_out, s] ----
        def proj_T(xT, bias_sb, dstT, wb0, wb1):
            # Two j-tiles accumulate in flight per i-sweep so the first matmul
            # only needs W block i=0 (not the full 2MB half) — cuts the
            # startup dead time while Wq streams in.
            for half, wch in ((0, wb0), (1, wb1)):
                for sh in range(2):
                    sl = slice(sh * 512, (sh + 1) * 512)
                    for jp in (0, 2):
                        ps0 = psc.tile([P, 512], F32, tag="pacc", name="ps0")
                        ps1 = psc.tile([P, 512], F32, tag="pacc", name="ps1")
                        for i in range(NDT):
                            nc.tensor.matmul(
                                ps0,
                                wch[:, i, jp * P : (jp + 1) * P],
                                xT[:, i, sl],
                                start=(i == 0),
                                stop=(i == NDT - 1),
                            )
                            nc.tensor.matmul(
                                ps1,
                                wch[:, i, (jp + 1) * P : (jp + 2) * P],
                                xT[:, i, sl],
                                start=(i == 0),
                                stop=(i == NDT - 1),
                            )
                        j = half * 4 + jp  # d_out tile == head index
                        nc.vector.tensor_scalar_add(
                            dstT[:, j, sl], ps0, bias_sb[:, j : j + 1]
                        )
                        nc.vector.tensor_scalar_add(
                            dstT[:, j + 1, sl], ps1, bias_sb[:, j + 1 : j + 2]
                        )

        # ---- v projection, natural [s, d_out], + bv, into vh_aug ----
        def proj_v_tile(vT, wch0, wch1, m):
                ps = psc.tile([P, S], F32, tag="pacc")
                for half, wch in ((0, wch0), (1, wch1)):
                    sl = slice(half * 512, (half + 1) * 512)
                    for i in range(NDT):
                        nc.tensor.matmul(
                            ps[:, sl],
                            vT[:, i, m * P : (m + 1) * P],
                            wch[:, i, :],
                            start=(i == 0),
                            stop=(i == NDT - 1),
                        )
                nc.vector.tensor_tensor(
                    vh_aug[:, :, m, 0:DB],
                    ps.rearrange("p (h n) -> p h n", n=DB),
                    bv_rep.rearrange("p (h n) -> p h n", n=DB),
                    OP.add,
                )

        # Gate MLP split in two pipelined stages: gates_b(h) runs one v-tile
        # after gates_a(h), so its psz matmuls never stall the in-order PE
        # queue waiting on the DVE tt product.
        def gates_a(h):
            # gx = kh@WgX + bgX  (matmul on PE, bias-add on DVE)
            psx = psc.tile([P, S], F32, tag="pacc")
            for sh in range(2):
                sl = slice(sh * 512, (sh + 1) * 512)
                nc.tensor.matmul(
                    psx[:, sl], WgX_sb, khT[:, h, sl], start=True, stop=True
                )
            gx = gpool.tile([P, S], adt, tag="gx", bufs=1)
            nc.vector.tensor_scalar_add(gx, psx, bgX_sb)
            # tt = (qh@WgY + bgY) * gx  in one DVE scalar_tensor_tensor
            psy = psc.tile([P, S], F32, tag="pacc")
            for sh in range(2):
                sl = slice(sh * 512, (sh + 1) * 512)
                nc.tensor.matmul(
                    psy[:, sl], WgY_sb, qhT[:, h, sl], start=True, stop=True
                )
            tt = gpool.tile([P, S], adt, tag="tt")
            nc.vector.scalar_tensor_tensor(
                tt, psy, bgY_sb, gx, OP.add, OP.mult
            )
            return tt

        def gates_b(h, tt):
            # z matmuls with replicated Wg2 columns: every output partition
            # carries the same gate row -> no cross-partition broadcast needed.
            for gi, dstT in ((0, khT), (1, qhT)):
                psz = psc.tile([P, S], F32, tag="pacc")
                for sh in range(2):
                    sl = slice(sh * 512, (sh + 1) * 512)
                    nc.tensor.matmul(
                        psz[:, sl], Wg2c[:, gi, :], tt[:, sl], start=True, stop=True
                    )
                g = gpool.tile([P, S], adt, tag=f"g{gi}", bufs=1)
                nc.scalar.activation(
                    g, psz, AF.Sigmoid, bias=bg2r[:, gi : gi + 1]
                )
                nc.vector.tensor_tensor(dstT[:, h, :], dstT[:, h, :], g, OP.mult)

        # ---- attention helpers (chunked so score matmuls + exp interleave
        # with other PE work instead of serializing behind ACT) ----
        def new_PT():
            return big.tile([P, KJ, S], adt, tag="bigslab", name="PT")

        def sc(h, PT, kjs):
            # scores (transposed) + exp -> P^T rows [s_k-in-tile, kj, q]
            for kj in kjs:
                ps = psc.tile([P, S], F32, tag="pacc")
                for sh in range(2):
                    sl = slice(sh * 512, (sh + 1) * 512)
                    nc.tensor.matmul(
                        ps[:, sl],
                        khT[:, h, kj * P : (kj + 1) * P],
                        qhT[:, h, sl],
                        start=True,
                        stop=True,
                    )
                nc.scalar.activation(
                    PT[:, kj, :], ps, AF.Exp,
                    bias=maskb[:, kj : kj + 1], scale=SCALE,
                )

        def pv_half(h, PT, pt2, qis):
            # PV with fused denominator; normalize; transpose into A_T
            for qi in qis:
                pv = ppv.tile([P, DB + 1], F32, tag="pv")
                for kj in range(KJ):
                    nc.tensor.matmul(
                        pv,
                        PT[:, kj, qi * P : (qi + 1) * P],
                        vh_aug[:, h, kj, :],
                        start=(kj == 0),
                        stop=(kj == KJ - 1),
                    )
                rec = smalls.tile([P, 1], F32, tag="rec")
                nc.vector.reciprocal(rec, pv[:, DB : DB + 1])
                asb = attp.tile([P, P], pdt, tag="asb")
                nc.vector.tensor_scalar_mul(asb, pv[:, 0:DB], rec)
                nc.tensor.transpose(
                    pt2[:, qi * P : (qi + 1) * P], asb, identp
                )
            if qis[-1] == KJ - 1:
                nc.vector.tensor_copy(A_T[:, h, :], pt2)

        def pv_block(h, PT):
            pt2 = ptr.tile([P, NDT * P], pdt, tag="trps")
            pv_half(h, PT, pt2, [0, 1, 2, 3])
            pv_half(h, PT, pt2, [4, 5, 6, 7])

        # ---- main phase schedule ----
        proj_T(xTq, bq_sb, qhT, wq0, wq1)
        wk0, wk1 = load_w(Wk, nc.scalar)
        xTk = load_xT(k, nc.scalar)
        proj_T(xTk, bk_sb, khT, wk0, wk1)

        wv0, wv1 = load_w(Wv, nc.scalar)
        xTv = load_xT(v, nc.scalar)

        # v projection with the gate MLP interleaved per s-tile, and the
        # early heads' scores+exp chunks spread across the loop so ACT's exp
        # stream (the attention bottleneck at ~9us/head) starts ~2 heads
        # early without ever stalling the in-order PE queue. PT slabs 0/1
        # reuse the xTq/xTk big-pool slots (dead by then); PT2 takes slot 3.
        PTs = {}
        tts = {}
        for m in range(KJ):
            proj_v_tile(xTv, wv0, wv1, m)
            tts[m] = gates_a(m)
            if m >= 1:
                gates_b(m - 1, tts.pop(m - 1))
            if m == 2:
                PTs[0] = new_PT()
                sc(0, PTs[0], [0, 1])
            elif m == 3:
                sc(0, PTs[0], [2, 3])
            elif m == 4:
                sc(0, PTs[0], [4, 5])
            elif m == 5:
                sc(0, PTs[0], [6, 7])
            elif m == 6:
                PTs[1] = new_PT()
                sc(1, PTs[1], [0, 1, 2])
            elif m == 7:
                sc(1, PTs[1], [3, 4, 5])
        gates_b(KJ - 1, tts.pop(KJ - 1))
        sc(1, PTs[1], [6, 7])
        PTs[2] = new_PT()
        sc(2, PTs[2], [0, 1, 2, 3])

        # Wm + bm streamed during the attention loop (gpsimd casts overlap
        # the ACT-bound exp stream).
        bm_rep = brep.tile([P, D], F32, tag="brep")
        with nc.allow_non_contiguous_dma(reason="tiny partition-major loads"):
            nc.gpsimd.dma_start(out=bm_rep, in_=bm[None, :].partition_broadcast(P))
        wm0, wm1 = load_w(Wm, nc.gpsimd)

        sc(2, PTs[2], [4, 5, 6, 7])

        # Attention: exp of head h (ACT) interleaves with PV of h-3 (PE),
        # chunk by chunk; the PT ring is 4 deep (PT(h) takes PT(h-4)'s slot).
        for h in range(3, H):
            hp = h - 3
            PTs[h] = new_PT()
            pt2 = ptr.tile([P, NDT * P], pdt, tag="trps")
            sc(h, PTs[h], [0, 1, 2, 3])
            pv_half(hp, PTs[hp], pt2, [0, 1, 2, 3])
            sc(h, PTs[h], [4, 5, 6, 7])
            pv_half(hp, PTs[hp], pt2, [4, 5, 6, 7])
            PTs.pop(hp)
        for h in range(H - 3, H):
            pv_block(h, PTs.pop(h))

        # ---- merge: out = A @ Wm + bm (evicted + stored in column halves,
        # alternating output DMA across the two HWDGE queues) ----
        for m in range(KJ):
            ps = psc.tile([P, S], F32, tag="pacc")
            for half, wch in ((0, wm0), (1, wm1)):
                sl = slice(half * 512, (half + 1) * 512)
                for i in range(NDT):
                    nc.tensor.matmul(
                        ps[:, sl],
                        A_T[:, i, m * P : (m + 1) * P],
                        wch[:, i, :],
                        start=(i == 0),
                        stop=(i == NDT - 1),
                    )
                osb = outp.tile([P, 512], F32, tag="osb")
                nc.vector.tensor_tensor(osb, ps[:, sl], bm_rep[:, sl], OP.add)
                deng = nc.sync if half == 0 else nc.scalar
                deng.dma_start(
                    out=out[m * P : (m + 1) * P, half * 512 : (half + 1) * 512],
                    in_=osb,
                )

    nc.finalize()
    return nc


_NC_CACHE = {}


def _get_nc(key=("bf16", "bf16")):
    if key not in _NC_CACHE:
        _NC_CACHE[key] = build_nc(
            proj_bf16=(key[0] == "bf16"), attn_bf16=(key[1] == "bf16")
        )
    return _NC_CACHE[key]


def _f32(a):
    return np.ascontiguousarray(np.asarray(a, dtype=np.float32))


def kernel(v, k, q, mask, Wv, bv, Wk, bk, Wq, bq, Wm, bm,
           WgX, bgX, WgY, bgY, Wg2, bg2):
    from concourse.bass_utils import run_bass_kernel_spmd

    nc = _get_nc()
    nb = int(np.asarray(q).shape[0])
    shared = {
        "Wq": _f32(Wq), "Wk": _f32(Wk), "Wv": _f32(Wv), "Wm": _f32(Wm),
        "bq": _f32(bq), "bk": _f32(bk), "bv": _f32(bv), "bm": _f32(bm),
        "WgX": _f32(WgX), "WgY": _f32(WgY), "Wg2": _f32(Wg2),
        "bgX": _f32(bgX), "bgY": _f32(bgY), "bg2": _f32(bg2),
    }
    in_maps = []
    for b in range(nb):
        m = dict(shared)
        m["q"] = _f32(q[b])
        m["k"] = _f32(k[b])
        m["v"] = _f32(v[b])
        m["mask"] = np.ascontiguousarray(
            np.asarray(mask[b], dtype=np.bool_).reshape(S).view(np.uint8)
        )
        in_maps.append(m)
    res = run_bass_kernel_spmd(nc, in_maps, list(range(nb)))
    return np.stack([res.results[b]["out"] for b in range(nb)]).astype(np.float32)


# revision 29
# speedup vs baseline: 1.1061x; 1.0756x over previous
"""Trainium2 Bass kernel for gated multi-head attention (nn_MHAtt_41274635714591).

Strategy: data-parallel over batch — 8 batches onto 8 NeuronCores, one batch per
core, no collectives. Per core (S=1024, D=1024, H=8, DB=128):

  1. Inputs f32->bf16 cast on ACT — NOT gpsimd (3.2x slower; was the top
     bottleneck: PE idled ~25us at each input phase start waiting on casts).
     128x128 transposes on PE -> xT [d, s].
  2. Projections (bf16 matmuls, fp32 PSUM): qhT/khT = (x @ W + b)^T via
     lhsT=W-colblock, rhs=xT; vh in natural [s, d] layout straight into
     vh_aug whose extra all-ones column yields the softmax denominator
     for free from the PV matmul. Weights stream as 2MB column-halves on
     the gpsimd DMA queue (so they never head-of-line-block the x rows on
     the sync queue); casts: Wq on DVE (startup), Wk/Wv on ACT (slack in
     the q/k phases), Wm on gpsimd (during the ACT-bound attention phase).
  3. Gate MLP per head: gx = psx + bgX on DVE; tt = (psy + bgY) * gx in one
     DVE scalar_tensor_tensor; sigmoid on ACT. Gate rows are produced already
     broadcast across partitions by replicating the Wg2 column across the
     matmul's stationary dim; gates multiply khT/qhT in place.
  4. Scores computed TRANSPOSED: S^T[k,q] = lhsT=khT-chunk, rhs=qhT.
     exp(scale*x + maskbias_k) on ACT writes P^T directly — no P transposes.
     The mask folds in as a per-partition additive -1e9 bias. Heads 0/1
     pulled into the v phase (m=1,3) and head 2 right after, so ACT's exp
     stream (the attention-phase bottleneck at ~8.9us/head) starts ~3 heads
     early.
  5. PV: out[q, 0:129] = sum_k P^T-chunk^T @ vh_aug; column 128 is the
     denominator; normalize with DVE reciprocal + tensor_scalar.
  6. att tiles transposed on PE into A_T [d, s]; merge matmul with Wm
     col-halves streamed during the attention loop; + bm; DMA out.

The harness calls kernel(**full_inputs); we shard batch across cores with
run_bass_kernel_spmd and stack the per-core outputs.
"""

import math
import os
import sys

for _p in ("/opt/trn_rl_repo", "/root/.axon_site/_ro/trn_rl_repo"):
    if os.path.isdir(_p) and _p not in sys.path:
        sys.path.insert(0, _p)

import numpy as np

import concourse.bass as bass
import concourse.mybir as mybir
import concourse.tile as tile
from concourse import bacc
from concourse.masks import make_identity

F32 = mybir.dt.float32
BF16 = mybir.dt.bfloat16
U8 = mybir.dt.uint8
AF = mybir.ActivationFunctionType
OP = mybir.AluOpType

B, S, D, H = 8, 1024, 1024, 8
DB = D // H          # 128 per-head dim
P = 128              # partitions
KJ = S // P          # 8 tiles of 128 along s
NDT = D // P         # 8 tiles of 128 along d
SCALE = 1.0 / math.sqrt(DB)
NEG = -1e9


def build_nc(proj_bf16=True, attn_bf16=True, repeat=1):
    """Emit the per-core program. repeat>1 wraps the whole body in a
    device-side loop (for timing)."""
    assert proj_bf16 and attn_bf16
    pdt = BF16
    adt = BF16
    # Bacc (not plain Bass): its compile pipeline fuses multi-sem waits into
    # event semaphores — this container's walrus rejects instructions carrying
    # more than one sync wait — and inserts GPSIMD library / ACT table loads.
    nc = bacc.Bacc()

    q = nc.dram_tensor("q", [S, D], F32, kind="ExternalInput")
    k = nc.dram_tensor("k", [S, D], F32, kind="ExternalInput")
    v = nc.dram_tensor("v", [S, D], F32, kind="ExternalInput")
    mask = nc.dram_tensor("mask", [S], U8, kind="ExternalInput")
    Wq = nc.dram_tensor("Wq", [D, D], F32, kind="ExternalInput")
    Wk = nc.dram_tensor("Wk", [D, D], F32, kind="ExternalInput")
    Wv = nc.dram_tensor("Wv", [D, D], F32, kind="ExternalInput")
    Wm = nc.dram_tensor("Wm", [D, D], F32, kind="ExternalInput")
    bq = nc.dram_tensor("bq", [D], F32, kind="ExternalInput")
    bk = nc.dram_tensor("bk", [D], F32, kind="ExternalInput")
    bv = nc.dram_tensor("bv", [D], F32, kind="ExternalInput")
    bm = nc.dram_tensor("bm", [D], F32, kind="ExternalInput")
    WgX = nc.dram_tensor("WgX", [DB, DB], F32, kind="ExternalInput")
    WgY = nc.dram_tensor("WgY", [DB, DB], F32, kind="ExternalInput")
    Wg2 = nc.dram_tensor("Wg2", [DB, 2], F32, kind="ExternalInput")
    bgX = nc.dram_tensor("bgX", [DB], F32, kind="ExternalInput")
    bgY = nc.dram_tensor("bgY", [DB], F32, kind="ExternalInput")
    bg2 = nc.dram_tensor("bg2", [2], F32, kind="ExternalInput")
    out = nc.dram_tensor("out", [S, D], F32, kind="ExternalOutput")

    from contextlib import ExitStack

    with tile.TileContext(nc) as tc, ExitStack() as ctx:
        consts = ctx.enter_context(tc.tile_pool(name="consts", bufs=1))
        persist = ctx.enter_context(tc.tile_pool(name="persist", bufs=1))
        big = ctx.enter_context(tc.tile_pool(name="big", bufs=4))
        xrow = ctx.enter_context(tc.tile_pool(name="xrow", bufs=5))
        xbrow = ctx.enter_context(tc.tile_pool(name="xbrow", bufs=2))
        wstream = ctx.enter_context(tc.tile_pool(name="wstream", bufs=3))
        wconv = ctx.enter_context(tc.tile_pool(name="wconv", bufs=2))
        gpool = ctx.enter_context(tc.tile_pool(name="gpool", bufs=2))
        attp = ctx.enter_context(tc.tile_pool(name="attp", bufs=2))
        smalls = ctx.enter_context(tc.tile_pool(name="smalls", bufs=2))
        outp = ctx.enter_context(tc.tile_pool(name="outp", bufs=2))
        brep = ctx.enter_context(tc.tile_pool(name="brep", bufs=1))
        # PSUM: psc 2x[128,1024]f32 (4 banks) + ppv 2x[128,129]f32 (2 banks)
        # + ptr 2x[128,1024]bf16 (2 banks) = 8 banks
        psc = ctx.enter_context(tc.tile_pool(name="psc", bufs=2, space="PSUM"))
        ppv = ctx.enter_context(tc.tile_pool(name="ppv", bufs=2, space="PSUM"))
        ptr = ctx.enter_context(tc.tile_pool(name="ptr", bufs=2, space="PSUM"))

        if repeat > 1:
            ctx.enter_context(tc.For_i(0, repeat, 1))

        # ---- identity + persistent activations ----
        identp = consts.tile([P, P], pdt, tag="identp")
        make_identity(nc, identp)

        qhT = persist.tile([P, H, S], adt, tag="qhT")   # [db, h, s] = (q@Wq+b)^T
        khT = persist.tile([P, H, S], adt, tag="khT")
        vh_aug = persist.tile([P, H, KJ, DB + 1], adt, tag="vh_aug")
        nc.vector.memset(vh_aug[:, :, :, DB : DB + 1], 1.0)
        A_T = persist.tile([P, H, S], pdt, tag="A_T")   # attention out, transposed

        def cast(eng, dst, src):
            if eng is nc.scalar:
                nc.scalar.copy(dst, src)
            else:
                eng.tensor_copy(dst, src)

        # ---- input transpose: x [s, d] -> xT [d-in-tile, i, s] (bf16) ----
        # Each 128-row block loads as TWO half-row DMAs fired simultaneously
        # on the sync and scalar HWDGE queues (per-queue descriptor
        # generation is the DMA latency driver, so splitting halves the
        # arrival time), one block ahead of the cast/transpose consumption.
        def load_xT(xdram, ceng):
            xT = big.tile([P, NDT, S], pdt, tag="bigslab")
            xfs = {}

            def issue(m):
                if m >= KJ:
                    return
                hs = []
                for half, deng in ((0, nc.sync), (1, nc.scalar)):
                    xf = xrow.tile([P, 512], F32, tag="xrow")
                    deng.dma_start(
                        out=xf,
                        in_=xdram[m * P : (m + 1) * P,
                                  half * 512 : (half + 1) * 512],
                    )
                    hs.append(xf)
                xfs[m] = hs

            issue(0)
            issue(1)
            for m in range(KJ):
                xb = xbrow.tile([P, D], pdt, tag="xbrow")
                halves = xfs.pop(m)
                pt = ptr.tile([P, NDT * P], pdt, tag="trps")
                # the two half-casts run on DIFFERENT engines concurrently,
                # halving the cast latency the transposes wait on
                for half in range(2):
                    sl = slice(half * 512, (half + 1) * 512)
                    cast(ceng if half == 0 else nc.vector,
                         xb[:, sl], halves[half])
                for half in range(2):
                    for dj in range(half * 4, half * 4 + 4):
                        nc.tensor.transpose(
                            pt[:, dj * P : (dj + 1) * P],
                            xb[:, dj * P : (dj + 1) * P],
                            identp,
                        )
                issue(m + 2)
                nc.vector.tensor_copy(
                    xT[:, :, m * P : (m + 1) * P],
                    pt.rearrange("p (a b) -> p a b", b=P),
                )
            return xT

        def load_w(Wdram, ceng):
            """Stream W in column-half order ([D, 512] then [D, 512]) as 2KB
            descriptor chunks on the gpsimd SWDGE queue: the projections'
            first accumulation sweep only needs half-0 chunk i=0, and half-1
            streams while half-0 is being consumed."""
            wbs = []
            for half in range(2):
                wb = wconv.tile(
                    [P, NDT, 512], pdt, tag=f"wchb{half}", name="wb"
                )
                wsrc = Wdram[:, half * 512 : (half + 1) * 512].rearrange(
                    "(i p) n -> p i n", p=P
                )
                for c in range(0, NDT, 2):
                    wf = wstream.tile([P, 2, 512], F32, tag="wch")
                    nc.gpsimd.dma_start(out=wf, in_=wsrc[:, c : c + 2, :])
                    cast(ceng, wb[:, c : c + 2, :], wf)
                wbs.append(wb)
            return wbs[0], wbs[1]

        # ---- startup: critical DMAs first (Wq chunks on the gpsimd queue,
        # q half-rows on the sync+scalar queues — all stream concurrently) ----
        wq0, wq1 = load_w(Wq, nc.vector)
        xTq = load_xT(q, nc.scalar)

        # ---- constants / small prep (nothing here is needed before ~15us;
        # emitted after the startup DMAs so the tiny SWDGE loads don't delay
        # them in the queues) ----
        with nc.allow_non_contiguous_dma(reason="tiny partition-major loads"):
            mask_u8 = consts.tile([P, KJ], U8, tag="mask_u8")
            nc.gpsimd.dma_start(
                out=mask_u8, in_=mask.rearrange("(o p) -> p o", p=P)
            )
            bq_sb = consts.tile([P, NDT], F32, tag="bq_sb")
            nc.gpsimd.dma_start(out=bq_sb, in_=bq.rearrange("(o p) -> p o", p=P))
            bk_sb = consts.tile([P, NDT], F32, tag="bk_sb")
            nc.gpsimd.dma_start(out=bk_sb, in_=bk.rearrange("(o p) -> p o", p=P))
            bgX_sb = consts.tile([P, 1], F32, tag="bgX_sb")
            nc.gpsimd.dma_start(out=bgX_sb, in_=bgX.rearrange("(o p) -> p o", p=P))
            bgY_sb = consts.tile([P, 1], F32, tag="bgY_sb")
            nc.gpsimd.dma_start(out=bgY_sb, in_=bgY.rearrange("(o p) -> p o", p=P))
            # bg2 replicated to every partition (activation bias must be [P, 1])
            bg2r = consts.tile([P, 2], F32, tag="bg2r")
            nc.gpsimd.dma_start(out=bg2r, in_=bg2[None, :].partition_broadcast(P))
            # free-axis bias bv, replicated across partitions (bm shares the
            # slot later — disjoint lifetimes)
            bv_rep = brep.tile([P, D], F32, tag="brep")
            nc.gpsimd.dma_start(out=bv_rep, in_=bv[None, :].partition_broadcast(P))
        maskb = consts.tile([P, KJ], F32, tag="maskb")
        nc.vector.tensor_scalar_mul(maskb, mask_u8, NEG)

        WgX_f = consts.tile([P, DB], F32, tag="WgX_f")
        nc.sync.dma_start(out=WgX_f, in_=WgX[:, :])
        WgY_f = consts.tile([P, DB], F32, tag="WgY_f")
        nc.sync.dma_start(out=WgY_f, in_=WgY[:, :])
        WgX_sb = consts.tile([P, DB], adt, tag="WgX_sb")
        nc.gpsimd.tensor_copy(WgX_sb, WgX_f)
        WgY_sb = consts.tile([P, DB], adt, tag="WgY_sb")
        nc.gpsimd.tensor_copy(WgY_sb, WgY_f)
        # Wg2 columns replicated across 128 stationary columns: the z matmul
        # then emits each gate row already broadcast over all 128 partitions.
        Wg2_f = consts.tile([P, 2], F32, tag="Wg2_f")
        nc.sync.dma_start(out=Wg2_f, in_=Wg2[:, :])
        Wg2c = consts.tile([P, 2, P], adt, tag="Wg2c")
        nc.vector.tensor_copy(Wg2c, Wg2_f[:, :, None].to_broadcast((P, 2, P)))

        # ---- q/k projections, output transposed [d_out, s] ----
        def proj_T(xT, bias_sb, dstT, wb0, wb1):
            # Two j-tiles accumulate in flight per i-sweep so the first matmul
            # only needs W block i=0 (not the full 2MB half) — cuts the
            # startup dead time while Wq streams in.
            for half, wch in ((0, wb0), (1, wb1)):
                for sh in range(2):
                    sl = slice(sh * 512, (sh + 1) * 512)
                    for jp in (0, 2):
                        ps0 = psc.tile([P, 512], F32, tag="pacc", name="ps0")
                        ps1 = psc.tile([P, 512], F32, tag="pacc", name="ps1")
                        for i in range(NDT):
                            nc.tensor.matmul(
                                ps0,
                                wch[:, i, jp * P : (jp + 1) * P],
                                xT[:, i, sl],
                                start=(i == 0),
                                stop=(i == NDT - 1),
                            )
                            nc.tensor.matmul(
                                ps1,
                                wch[:, i, (jp + 1) * P : (jp + 2) * P],
                                xT[:, i, sl],
                                start=(i == 0),
                                stop=(i == NDT - 1),
                            )
                        j = half * 4 + jp  # d_out tile == head index
                        nc.vector.tensor_scalar_add(
                            dstT[:, j, sl], ps0, bias_sb[:, j : j + 1]
                        )
                        nc.vector.tensor_scalar_add(
                            dstT[:, j + 1, sl], ps1, bias_sb[:, j + 1 : j + 2]
                        )

        # ---- v projection, natural [s, d_out], + bv, into vh_aug ----
        def proj_v_tile(vT, wch0, wch1, m):
                ps = psc.tile([P, S], F32, tag="pacc")
                for half, wch in ((0, wch0), (1, wch1)):
                    sl = slice(half * 512, (half + 1) * 512)
                    for i in range(NDT):
                        nc.tensor.matmul(
                            ps[:, sl],
                            vT[:, i, m * P : (m + 1) * P],
                            wch[:, i, :],
                            start=(i == 0),
                            stop=(i == NDT - 1),
                        )
                nc.vector.tensor_tensor(
                    vh_aug[:, :, m, 0:DB],
                    ps.rearrange("p (h n) -> p h n", n=DB),
                    bv_rep.rearrange("p (h n) -> p h n", n=DB),
                    OP.add,
                )

        # Gate MLP split in two pipelined stages: gates_b(h) runs one v-tile
        # after gates_a(h), so its psz matmuls never stall the in-order PE
        # queue waiting on the DVE tt product.
        def gates_a(h):
            # gx = kh@WgX + bgX  (matmul on PE, bias-add on DVE)
            psx = psc.tile([P, S], F32, tag="pacc")
            for sh in range(2):
                sl = slice(sh * 512, (sh + 1) * 512)
                nc.tensor.matmul(
                    psx[:, sl], WgX_sb, khT[:, h, sl], start=True, stop=True
                )
            gx = gpool.tile([P, S], adt, tag="gx", bufs=1)
            nc.vector.tensor_scalar_add(gx, psx, bgX_sb)
            # tt = (qh@WgY + bgY) * gx  in one DVE scalar_tensor_tensor
            psy = psc.tile([P, S], F32, tag="pacc")
            for sh in range(2):
                sl = slice(sh * 512, (sh + 1) * 512)
                nc.tensor.matmul(
                    psy[:, sl], WgY_sb, qhT[:, h, sl], start=True, stop=True
                )
            tt = gpool.tile([P, S], adt, tag="tt")
            nc.vector.scalar_tensor_tensor(
                tt, psy, bgY_sb, gx, OP.add, OP.mult
            )
            return tt

        def gates_b(h, tt):
            # z matmuls with replicated Wg2 columns: every output partition
            # carries the same gate row -> no cross-partition broadcast needed.
            for gi, dstT in ((0, khT), (1, qhT)):
                psz = psc.tile([P, S], F32, tag="pacc")
                for sh in range(2):
                    sl = slice(sh * 512, (sh + 1) * 512)
                    nc.tensor.matmul(
                        psz[:, sl], Wg2c[:, gi, :], tt[:, sl], start=True, stop=True
                    )
                g = gpool.tile([P, S], adt, tag=f"g{gi}", bufs=1)
                nc.scalar.activation(
                    g, psz, AF.Sigmoid, bias=bg2r[:, gi : gi + 1]
                )
                nc.vector.tensor_tensor(dstT[:, h, :], dstT[:, h, :], g, OP.mult)

        # ---- attention helpers (chunked so score matmuls + exp interleave
        # with other PE work instead of serializing behind ACT) ----
        def new_PT():
            return big.tile([P, KJ, S], adt, tag="bigslab", name="PT")

        def sc(h, PT, kjs):
            # scores (transposed) + exp -> P^T rows [s_k-in-tile, kj, q]
            for kj in kjs:
                ps = psc.tile([P, S], F32, tag="pacc")
                for sh in range(2):
                    sl = slice(sh * 512, (sh + 1) * 512)
                    nc.tensor.matmul(
                        ps[:, sl],
                        khT[:, h, kj * P : (kj + 1) * P],
                        qhT[:, h, sl],
                        start=True,
                        stop=True,
                    )
                nc.scalar.activation(
                    PT[:, kj, :], ps, AF.Exp,
                    bias=maskb[:, kj : kj + 1], scale=SCALE,
                )

        def pv_half(h, PT, pt2, qis):
            # PV with fused denominator; normalize; transpose into A_T
            for qi in qis:
                pv = ppv.tile([P, DB + 1], F32, tag="pv")
                for kj in range(KJ):
                    nc.tensor.matmul(
                        pv,
                        PT[:, kj, qi * P : (qi + 1) * P],
                        vh_aug[:, h, kj, :],
                        start=(kj == 0),
                        stop=(kj == KJ - 1),
                    )
                rec = smalls.tile([P, 1], F32, tag="rec")
                nc.vector.reciprocal(rec, pv[:, DB : DB + 1])
                asb = attp.tile([P, P], pdt, tag="asb")
                nc.vector.tensor_scalar_mul(asb, pv[:, 0:DB], rec)
                nc.tensor.transpose(
                    pt2[:, qi * P : (qi + 1) * P], asb, identp
                )
            if qis[-1] == KJ - 1:
                nc.vector.tensor_copy(A_T[:, h, :], pt2)

        def pv_block(h, PT):
            pt2 = ptr.tile([P, NDT * P], pdt, tag="trps")
            pv_half(h, PT, pt2, [0, 1, 2, 3])
            pv_half(h, PT, pt2, [4, 5, 6, 7])

        # ---- main phase schedule ----
        proj_T(xTq, bq_sb, qhT, wq0, wq1)
        wk0, wk1 = load_w(Wk, nc.scalar)
        xTk = load_xT(k, nc.scalar)
        proj_T(xTk, bk_sb, khT, wk0, wk1)

        wv0, wv1 = load_w(Wv, nc.scalar)
        xTv = load_xT(v, nc.scalar)

        # v projection with the gate MLP interleaved per s-tile, and the
        # early heads' scores+exp chunks spread across the loop so ACT's exp
        # stream (the attention bottleneck at ~9us/head) starts ~2 heads
        # early without ever stalling the in-order PE queue. PT slabs 0/1
        # reuse the xTq/xTk big-pool slots (dead by then); PT2 takes slot 3.
        PTs = {}
        tts = {}
        for m in range(KJ):
            proj_v_tile(xTv, wv0, wv1, m)
            tts[m] = gates_a(m)
            if m >= 1:
                gates_b(m - 1, tts.pop(m - 1))
            if m == 2:
                PTs[0] = new_PT()
                sc(0, PTs[0], [0, 1])
            elif m == 3:
                sc(0, PTs[0], [2, 3])
            elif m == 4:
                sc(0, PTs[0], [4, 5])
            elif m == 5:
                sc(0, PTs[0], [6, 7])
            elif m == 6:
                PTs[1] = new_PT()
                sc(1, PTs[1], [0, 1, 2])
            elif m == 7:
                sc(1, PTs[1], [3, 4, 5])
        gates_b(KJ - 1, tts.pop(KJ - 1))
        sc(1, PTs[1], [6, 7])
        PTs[2] = new_PT()
        sc(2, PTs[2], [0, 1, 2, 3])

        # Wm + bm streamed during the attention loop (gpsimd casts overlap
        # the ACT-bound exp stream).
        bm_rep = brep.tile([P, D], F32, tag="brep")
        with nc.allow_non_contiguous_dma(reason="tiny partition-major loads"):
            nc.gpsimd.dma_start(out=bm_rep, in_=bm[None, :].partition_broadcast(P))
        wm0, wm1 = load_w(Wm, nc.gpsimd)

        sc(2, PTs[2], [4, 5, 6, 7])

        # Attention: exp of head h (ACT) interleaves with PV of h-3 (PE),
        # chunk by chunk; the PT ring is 4 deep (PT(h) takes PT(h-4)'s slot).
        for h in range(3, H):
            hp = h - 3
            PTs[h] = new_PT()
            pt2 = ptr.tile([P, NDT * P], pdt, tag="trps")
            sc(h, PTs[h], [0, 1, 2, 3])
            pv_half(hp, PTs[hp], pt2, [0, 1, 2, 3])
            sc(h, PTs[h], [4, 5, 6, 7])
            pv_half(hp, PTs[hp], pt2, [4, 5, 6, 7])
            PTs.pop(hp)
        for h in range(H - 3, H):
            pv_block(h, PTs.pop(h))

        # ---- merge: out = A @ Wm + bm (evicted + stored in column halves,
        # alternating output DMA across the two HWDGE queues) ----
        for m in range(KJ):
            ps = psc.tile([P, S], F32, tag="pacc")
            for half, wch in ((0, wm0), (1, wm1)):
                sl = slice(half * 512, (half + 1) * 512)
                for i in range(NDT):
                    nc.tensor.matmul(
                        ps[:, sl],
                        A_T[:, i, m * P : (m + 1) * P],
                        wch[:, i, :],
                        start=(i == 0),
                        stop=(i == NDT - 1),
                    )
                osb = outp.tile([P, 512], F32, tag="osb")
                nc.vector.tensor_tensor(osb, ps[:, sl], bm_rep[:, sl], OP.add)
                deng = nc.sync if half == 0 else nc.scalar
                deng.dma_start(
                    out=out[m * P : (m + 1) * P, half * 512 : (half + 1) * 512],
                    in_=osb,
                )

    nc.finalize()
    return nc


_NC_CACHE = {}


def _get_nc(key=("bf16", "bf16")):
    if key not in _NC_CACHE:
        _NC_CACHE[key] = build_nc(
            proj_bf16=(key[0] == "bf16"), attn_bf16=(key[1] == "bf16")
        )
    return _NC_CACHE[key]


def _f32(a):
    return np.ascontiguousarray(np.asarray(a, dtype=np.float32))


def kernel(v, k, q, mask, Wv, bv, Wk, bk, Wq, bq, Wm, bm,
           WgX, bgX, WgY, bgY, Wg2, bg2):
    from concourse.bass_utils import run_bass_kernel_spmd

    nc = _get_nc()
    nb = int(np.asarray(q).shape[0])
    shared = {
        "Wq": _f32(Wq), "Wk": _f32(Wk), "Wv": _f32(Wv), "Wm": _f32(Wm),
        "bq": _f32(bq), "bk": _f32(bk), "bv": _f32(bv), "bm": _f32(bm),
        "WgX": _f32(WgX), "WgY": _f32(WgY), "Wg2": _f32(Wg2),
        "bgX": _f32(bgX), "bgY": _f32(bgY), "bg2": _f32(bg2),
    }
    in_maps = []
    for b in range(nb):
        m = dict(shared)
        m["q"] = _f32(q[b])
        m["k"] = _f32(k[b])
        m["v"] = _f32(v[b])
        m["mask"] = np.ascontiguousarray(
            np.asarray(mask[b], dtype=np.bool_).reshape(S).view(np.uint8)
        )
        in_maps.append(m)
    res = run_bass_kernel_spmd(nc, in_maps, list(range(nb)))
    return np.stack([res.results[b]["out"] for b in range(nb)]).astype(np.float32)
